# revision 1
# baseline (speedup 1.0000x reference)
"""Trainium2 Bass kernel for nn_GNO2d (spectral conv, method-25 branch).

Sharded over pipes P=8, one pipe per NeuronCore. Per pipe the computation is a
truncated 2-D rFFT -> per-mode complex channel mixing -> inverse rFFT,
implemented entirely as TensorEngine matmuls against small DFT constant
matrices (bf16 operands, fp32 PSUM accumulation):

  A: Z[kx,w]   = sum_h x[h,w] e^{-i th_kx h}          (64 retained kx rows)
  B: F[kx,ky]  = sum_w Z[kx,w] e^{-i ph_ky w}         (32 retained ky cols)
  T1: PE-transpose F from [kx,..] to [(rW,i),..] layout
  MIX: f[o,..] = sum_i W[i,o] F[i,..]  (complex, via K=(re/im,i)=64 matmuls)
  T2: PE-transpose f to [(kx,rF),..] layout
  D: U[.,h]    = sum_{kx,rF} f e^{+i th h}            (complex combine folded
  T3: PE-transpose U to [(hp,ky,rU),..] layout          into constant matrices)
  E: y[.,w]    = Re sum_{ky,rU} c_ky U e^{+i ph w} / (H W)
"""

import numpy as np
import ml_dtypes
from contextlib import ExitStack

import concourse.bass as bass
import concourse.tile as tile
import concourse.mybir as mybir
from concourse import bacc
from concourse.bass_utils import run_bass_kernel_spmd

P, B, C, H, W = 8, 4, 32, 256, 256
M1, M2 = 32, 32
KX = 2 * M1  # 64 retained kx rows
FP = mybir.dt.float32
BF = mybir.dt.bfloat16
BFNP = ml_dtypes.bfloat16


def _constants():
    """Host-side DFT constant matrices, bf16."""
    freqs = np.concatenate([np.arange(M1), np.arange(H - M1, H)])  # kx freqs
    th = 2 * np.pi * np.outer(np.arange(H), freqs) / H             # [H, KX]
    phi = 2 * np.pi * np.outer(np.arange(W), np.arange(M2)) / W    # [W, M2]

    fh = np.concatenate([np.cos(th), -np.sin(th)], axis=1)         # [256, 128]
    cwa = np.concatenate([np.cos(phi), -np.sin(phi)], axis=1)      # [256, 64]
    cwb = np.concatenate([np.sin(phi), np.cos(phi)], axis=1)       # [256, 64]

    # EH2 rows (kx*2 + rF), cols (rU*256 + h)
    eh = np.zeros((128, 512), np.float32)
    c, s = np.cos(th.T), np.sin(th.T)                              # [KX, H]
    eh[0::2, 0:256] = c      # (kx,re)->(re,h): +cos
    eh[0::2, 256:512] = s    # (kx,re)->(im,h): +sin
    eh[1::2, 0:256] = -s     # (kx,im)->(re,h): -sin
    eh[1::2, 256:512] = c    # (kx,im)->(im,h): +cos

    # EW2 rows (hp*64 + ky*2 + rU) [hp halves duplicated], cols w
    cky = np.where(np.arange(M2) == 0, 1.0, 2.0)[:, None]
    ewc = cky * np.cos(phi.T) / (H * W)                            # [M2, W]
    ews = cky * np.sin(phi.T) / (H * W)
    ew = np.zeros((128, 256), np.float32)
    ew[0:64:2] = ewc
    ew[1:64:2] = -ews
    ew[64:128] = ew[0:64]

    consts = {
        "FH": fh, "CWA": cwa, "CWB": cwb, "EH2": eh, "EW2": ew,
        "ID32": np.eye(32, dtype=np.float32),
        "ID64": np.eye(64, dtype=np.float32),
        "ID128": np.eye(128, dtype=np.float32),
    }
    return {k: np.ascontiguousarray(v.astype(BFNP)) for k, v in consts.items()}


def _build():
    nc = bacc.Bacc("TRN2", target_bir_lowering=False, debug=False, num_devices=P)
    xp = nc.dram_tensor("xp", [B, C, H, W], FP, kind="ExternalInput").ap()
    w1re = nc.dram_tensor("w1re", [C, C, M1, M2], FP, kind="ExternalInput").ap()
    w1im = nc.dram_tensor("w1im", [C, C, M1, M2], FP, kind="ExternalInput").ap()
    w4re = nc.dram_tensor("w4re", [C, C, M1, M2], FP, kind="ExternalInput").ap()
    w4im = nc.dram_tensor("w4im", [C, C, M1, M2], FP, kind="ExternalInput").ap()
    cdr = {}
    for name, shape in [("FH", [256, 128]), ("CWA", [256, 64]), ("CWB", [256, 64]),
                        ("EH2", [128, 512]), ("EW2", [128, 256]),
                        ("ID32", [32, 32]), ("ID64", [64, 64]), ("ID128", [128, 128])]:
        cdr[name] = nc.dram_tensor(name, shape, BF, kind="ExternalInput").ap()
    yp = nc.dram_tensor("yp", [B, C, H, W], FP, kind="ExternalOutput").ap()

    with tile.TileContext(nc) as tc, ExitStack() as ctx:
        # -------- constants to SBUF --------
        cp = ctx.enter_context(tc.tile_pool(name="consts", bufs=1))
        fh_sb, cwa_sb, cwb_sb = [], [], []
        for hb in range(2):
            t = cp.tile([128, 128], BF, tag=f"fh{hb}")
            nc.sync.dma_start(t[:], cdr["FH"][hb * 128:(hb + 1) * 128, :])
            fh_sb.append(t)
            ta = cp.tile([128, 64], BF, tag=f"cwa{hb}")
            nc.sync.dma_start(ta[:], cdr["CWA"][hb * 128:(hb + 1) * 128, :])
            cwa_sb.append(ta)
            tb = cp.tile([128, 64], BF, tag=f"cwb{hb}")
            nc.sync.dma_start(tb[:], cdr["CWB"][hb * 128:(hb + 1) * 128, :])
            cwb_sb.append(tb)
        eh_sb = cp.tile([128, 512], BF, tag="eh")
        nc.sync.dma_start(eh_sb[:], cdr["EH2"][:])
        ew_sb = cp.tile([128, 256], BF, tag="ew")
        nc.sync.dma_start(ew_sb[:], cdr["EW2"][:])
        id32 = cp.tile([32, 32], BF, tag="id32")
        nc.sync.dma_start(id32[:], cdr["ID32"][:])
        id64 = cp.tile([64, 64], BF, tag="id64")
        nc.sync.dma_start(id64[:], cdr["ID64"][:])
        id128 = cp.tile([128, 128], BF, tag="id128")
        nc.sync.dma_start(id128[:], cdr["ID128"][:])

        # -------- persistent intermediate tensors --------
        big = ctx.enter_context(tc.tile_pool(name="big", bufs=1))
        # Fbig cols: b*4096 + rF*2048 + ky*64 + rW*32 + i ; rows kx64
        fbig = big.tile([64, 16384], BF, tag="fbig")
        # FT cols: kx*256 + ky*8 + rF*4 + b ; rows (rW2, i32)
        ft = big.tile([64, 16384], BF, tag="ft")
        # P4 cols: ky*512 + b*128 + kx*2 + rF ; rows o32
        p4 = big.tile([32, 16384], BF, tag="p4")
        # TD cols: ky*128 + b*32 + o ; rows (kx64, rF2) interleaved kx*2+rF
        td = big.tile([128, 4096], BF, tag="td")
        # U cols: hh*128 + hp*64 + ky*2 + rU ; rows (b4, o32)
        u_sb = big.tile([128, 16384], BF, tag="u")

        # ================= Phase 1: stages A + B =================
        with ExitStack() as p1:
            xpool = p1.enter_context(tc.tile_pool(name="x", bufs=3))
            ztpool = p1.enter_context(tc.tile_pool(name="zt", bufs=4))
            psa = p1.enter_context(tc.tile_pool(name="psa", bufs=2, space="PSUM"))
            psb = p1.enter_context(tc.tile_pool(name="psb", bufs=2, space="PSUM"))
            for b in range(B):
                for i in range(C):
                    xt = xpool.tile([128, 2, 256], BF, tag="xt")
                    nc.gpsimd.dma_start(
                        xt[:], xp[b, i].rearrange("(hb hp) w -> hp hb w", hb=2))
                    zts = []
                    for ws in range(2):
                        pa = psa.tile([128, 128], FP, tag="pa")
                        for hb in range(2):
                            nc.tensor.matmul(
                                pa[:],
                                xt[:, hb, ws * 128: ws * 128 + 128],
                                fh_sb[hb][:],
                                start=(hb == 0), stop=(hb == 1))
                        zt = ztpool.tile([128, 128], BF, tag="zt")
                        nc.vector.tensor_copy(zt[:], pa[:])
                        zts.append(zt)
                    pb = psb.tile([64, 64], FP, tag="pb")
                    for ws in range(2):
                        nc.tensor.matmul(pb[:], zts[ws][:, 0:64], cwa_sb[ws][:],
                                         start=(ws == 0), stop=False)
                        nc.tensor.matmul(pb[:], zts[ws][:, 64:128], cwb_sb[ws][:],
                                         start=False, stop=(ws == 1))
                    # Fbig col = b*4096 + rF*2048 + ky*64 + rW*32 + i
                    fb6 = fbig[:].rearrange(
                        "p (b rf ky rw i) -> p b rf ky rw i",
                        b=B, rf=2, ky=M2, rw=2, i=C)
                    # rW=0 rows of FT: [Fre | Fim]
                    nc.vector.tensor_copy(
                        fb6[:, b, :, :, 0, i],
                        pb[:].rearrange("p (rf ky) -> p rf ky", rf=2))
                    # rW=1 rows of FT: [-Fim | Fre]
                    nc.scalar.mul(fb6[:, b, 0, :, 1, i], pb[:, 32:64], -1.0)
                    nc.scalar.copy(fb6[:, b, 1, :, 1, i], pb[:, 0:32])

        # ================= Phase T1: F -> FT transpose =================
        ft5 = ft[:].rearrange("p (kx ky rf b) -> p kx ky rf b",
                              kx=KX, ky=M2, rf=2, b=B)
        with ExitStack() as pt1:
            pst = pt1.enter_context(tc.tile_pool(name="pst", bufs=4, space="PSUM"))
            for b in range(B):
                for rf in range(2):
                    for ky in range(M2):
                        c0 = b * 4096 + rf * 2048 + ky * 64
                        pt = pst.tile([64, 64], BF, tag="pt")
                        nc.tensor.transpose(pt[:], fbig[:, c0:c0 + 64], id64[:])
                        nc.vector.tensor_copy(ft5[:, :, ky, rf, b], pt[:])

        # ================= Phase MIX =================
        with ExitStack() as pm:
            wpool = pm.enter_context(tc.tile_pool(name="w", bufs=2))
            psm = pm.enter_context(tc.tile_pool(name="psm", bufs=4, space="PSUM"))
            for q in range(16):  # kx quad
                wt = wpool.tile([64, 4096], BF, tag="wt")  # cols o*128+kxs*32+ky
                if q < 8:
                    sre, sim, kxo = w1re, w1im, q * 4
                else:
                    sre, sim, kxo = w4re, w4im, (q - 8) * 4
                nc.gpsimd.dma_start(
                    wt[0:32, :], sre[:, :, kxo:kxo + 4, :])
                nc.gpsimd.dma_start(
                    wt[32:64, :], sim[:, :, kxo:kxo + 4, :])
                wt4 = wt[:].rearrange("p (o kxs ky) -> p o kxs ky", o=C, kxs=4)
                p45 = p4[:].rearrange("p (ky b kx rf) -> p ky rf b kx",
                                      ky=M2, b=B, kx=KX)
                for kxs in range(4):
                    kx = q * 4 + kxs
                    pm_t = psm.tile([32, 256], FP, tag="pmix")
                    for ky in range(M2):
                        nc.tensor.matmul(
                            pm_t[:, ky * 8:ky * 8 + 8],
                            wt4[:, :, kxs, ky],               # [64, 32]
                            ft[:, kx * 256 + ky * 8:kx * 256 + ky * 8 + 8],
                            start=True, stop=True)
                    # psum cols (ky, rF, b) -> P4 col ky*512 + b*128 + kx*2 + rF
                    nc.vector.tensor_copy(
                        p45[:, :, :, :, kx],
                        pm_t[:].rearrange("p (ky rf b) -> p ky rf b",
                                          ky=M2, rf=2))

        # ================= Phase T2 + D =================
        with ExitStack() as pt2:
            pst2 = pt2.enter_context(tc.tile_pool(name="pst2", bufs=4, space="PSUM"))
            for ky in range(M2):
                for b in range(B):
                    c0 = ky * 512 + b * 128
                    pt = pst2.tile([128, 32], BF, tag="pt2")
                    nc.tensor.transpose(pt[:], p4[:, c0:c0 + 128], id32[:])
                    nc.vector.tensor_copy(
                        td[:, ky * 128 + b * 32:ky * 128 + b * 32 + 32], pt[:])
        with ExitStack() as pd_s:
            psd = pd_s.enter_context(tc.tile_pool(name="psd", bufs=3, space="PSUM"))
            u4 = u_sb[:].rearrange("p (hh hp ky ru) -> p ru hh hp ky",
                                   hh=128, hp=2, ky=M2)
            for ky in range(M2):
                pd = psd.tile([128, 512], FP, tag="pd")
                nc.tensor.matmul(pd[:], td[:, ky * 128:(ky + 1) * 128],
                                 eh_sb[:], start=True, stop=True)
                # psum cols (rU2, h256) -> U col hh*128 + hp*64 + ky*2 + rU
                nc.vector.tensor_copy(
                    u4[:, :, :, :, ky],
                    pd[:].rearrange("p (ru hh hp) -> p ru hh hp", ru=2, hh=128))

        # ================= Phase T3 + E + output =================
        yb = yp.rearrange("b o h w -> (b o) h w")
        with ExitStack() as pe_s:
            pst3 = pe_s.enter_context(tc.tile_pool(name="pst3", bufs=3, space="PSUM"))
            utp = pe_s.enter_context(tc.tile_pool(name="ut", bufs=3))
            pse = pe_s.enter_context(tc.tile_pool(name="pse", bufs=4, space="PSUM"))
            ostp = pe_s.enter_context(tc.tile_pool(name="ost", bufs=3))
            ost = None
            for hh in range(128):
                pt = pst3.tile([128, 128], BF, tag="pt3")
                nc.tensor.transpose(pt[:], u_sb[:, hh * 128:(hh + 1) * 128],
                                    id128[:])
                ut = utp.tile([128, 128], BF, tag="ut")
                nc.vector.tensor_copy(ut[:], pt[:])
                for hp in range(2):
                    h = hh * 2 + hp
                    if h % 8 == 0:
                        ost = ostp.tile([128, 2048], FP, tag="ost")
                    pe = pse.tile([128, 256], FP, tag="pe")
                    nc.tensor.matmul(pe[:], ut[hp * 64:(hp + 1) * 64, :],
                                     ew_sb[hp * 64:(hp + 1) * 64, :],
                                     start=True, stop=True)
                    eng = nc.vector if h % 2 == 0 else nc.scalar
                    if h % 2 == 0:
                        nc.vector.tensor_copy(
                            ost[:, (h % 8) * 256:(h % 8) * 256 + 256], pe[:])
                    else:
                        nc.scalar.copy(
                            ost[:, (h % 8) * 256:(h % 8) * 256 + 256], pe[:])
                    if h % 8 == 7:
                        nc.sync.dma_start(yb[:, h - 7:h + 1, :], ost[:])

    nc.compile()
    return nc


_NC = None


def kernel(x, w1_re, w1_im, w4_re, w4_im):
    global _NC
    if _NC is None:
        _NC = _build()
    consts = _constants()
    in_maps = []
    for p in range(P):
        m = {
            "xp": np.ascontiguousarray(x[p], dtype=np.float32),
            "w1re": np.ascontiguousarray(w1_re[:, :, p]),
            "w1im": np.ascontiguousarray(w1_im[:, :, p]),
            "w4re": np.ascontiguousarray(w4_re[:, :, p]),
            "w4im": np.ascontiguousarray(w4_im[:, :, p]),
        }
        m.update(consts)
        in_maps.append(m)
    res = run_bass_kernel_spmd(_NC, in_maps, core_ids=list(range(P)))
    return np.stack([res.results[p]["yp"] for p in range(P)], axis=0)


if __name__ == "__main__":
    rng = np.random.default_rng(0)
    x = rng.standard_normal((P, B, C, H, W)).astype(np.float32)
    wshape = (C, C, P, M1, M2)
    ws = [(rng.random(wshape, np.float32) / (C * C)).astype(np.float32)
          for _ in range(4)]
    out = kernel(x, *ws)
    print("out", out.shape, out.dtype, float(np.abs(out).max()))



# revision 22
# speedup vs baseline: 1.1192x; 1.1192x over previous
"""Trainium2 Bass kernel for nn_GNO2d (spectral conv, method-25 branch).

Sharded over pipes P=8, one pipe per NeuronCore. Per pipe the computation is a
truncated 2-D rFFT -> per-mode complex channel mixing -> inverse rFFT,
implemented entirely as TensorEngine matmuls against small DFT constant
matrices (bf16 operands, fp32 PSUM accumulation):

  A: Z[kx,w]   = sum_h x[h,w] e^{-i th_kx h}          (64 retained kx rows)
  B: F[kx,ky]  = sum_w Z[kx,w] e^{-i ph_ky w}         (32 retained ky cols)
  T1: PE-transpose F from [kx,..] to [(rW,i),..] layout
  MIX: f[o,..] = sum_i W[i,o] F[i,..]  (complex, via K=(re/im,i)=64 matmuls)
  T2: PE-transpose f to [(kx,rF),..] layout
  D: U[.,h]    = sum_{kx,rF} f e^{+i th h}            (complex combine folded
  T3: PE-transpose U to [(hp,ky,rU),..] layout          into constant matrices)
  E: y[.,w]    = Re sum_{ky,rU} c_ky U e^{+i ph w} / (H W)
"""

import numpy as np
import ml_dtypes
from contextlib import ExitStack

import concourse.bass as bass
import concourse.tile as tile
import concourse.mybir as mybir
from concourse import bacc
from concourse.bass_utils import run_bass_kernel_spmd

P, B, C, H, W = 8, 4, 32, 256, 256
M1, M2 = 32, 32
KX = 2 * M1  # 64 retained kx rows
FP = mybir.dt.float32
BF = mybir.dt.bfloat16
BFNP = ml_dtypes.bfloat16


def _constants():
    """Host-side DFT constant matrices, bf16."""
    freqs = np.concatenate([np.arange(M1), np.arange(H - M1, H)])  # kx freqs
    th = 2 * np.pi * np.outer(np.arange(H), freqs) / H             # [H, KX]
    phi = 2 * np.pi * np.outer(np.arange(W), np.arange(M2)) / W    # [W, M2]

    fh = np.concatenate([np.cos(th), -np.sin(th)], axis=1)         # [256, 128]
    cwa = np.concatenate([np.cos(phi), -np.sin(phi)], axis=1)      # [256, 64]
    cwb = np.concatenate([np.sin(phi), np.cos(phi)], axis=1)       # [256, 64]

    # EH2 rows (kx*2 + rF), cols (hh*4 + hp*2 + rU)  [h = hh*2 + hp]
    eh = np.zeros((128, 512), np.float32)
    c, s = np.cos(th.T), np.sin(th.T)                              # [KX, H]
    eh[0::2, 0:256] = c      # (kx,re)->(re,h): +cos
    eh[0::2, 256:512] = s    # (kx,re)->(im,h): +sin
    eh[1::2, 0:256] = -s     # (kx,im)->(re,h): -sin
    eh[1::2, 256:512] = c    # (kx,im)->(im,h): +cos
    # reorder cols from (rU, hh, hp) to (hh, hp, rU)
    eh = np.ascontiguousarray(
        eh.reshape(128, 2, 128, 2).transpose(0, 2, 3, 1).reshape(128, 512))

    # EW3 rows (hp*64 + ky*2 + rU), cols (hp'*256 + w), block-diagonal in hp
    cky = np.where(np.arange(M2) == 0, 1.0, 2.0)[:, None]
    ewc = cky * np.cos(phi.T) / (H * W)                            # [M2, W]
    ews = cky * np.sin(phi.T) / (H * W)
    ew = np.zeros((128, 512), np.float32)
    for hp in range(2):
        ew[hp * 64:hp * 64 + 64:2, hp * 256:hp * 256 + 256] = ewc
        ew[hp * 64 + 1:hp * 64 + 64:2, hp * 256:hp * 256 + 256] = -ews

    consts = {
        "FH": fh, "CWA": cwa, "CWB": cwb, "EH2": eh, "EW3": ew,
        "ID32": np.eye(32, dtype=np.float32),
        "ID64": np.eye(64, dtype=np.float32),
        "ID128": np.eye(128, dtype=np.float32),
    }
    return {k: np.ascontiguousarray(v.astype(BFNP)) for k, v in consts.items()}


F16 = mybir.dt.float16


def _build():
    nc = bacc.Bacc("TRN2", target_bir_lowering=False, debug=False, num_devices=P)
    xp = nc.dram_tensor("xp", [B, C, H, W], BF, kind="ExternalInput").ap()
    w1re = nc.dram_tensor("w1re", [C, C, M1, M2], BF, kind="ExternalInput").ap()
    w1im = nc.dram_tensor("w1im", [C, C, M1, M2], BF, kind="ExternalInput").ap()
    w4re = nc.dram_tensor("w4re", [C, C, M1, M2], BF, kind="ExternalInput").ap()
    w4im = nc.dram_tensor("w4im", [C, C, M1, M2], BF, kind="ExternalInput").ap()
    cdr = {}
    for name, shape in [("FH", [256, 128]), ("CWA", [256, 64]), ("CWB", [256, 64]),
                        ("EH2", [128, 512]), ("EW3", [128, 512]),
                        ("ID32", [32, 32]), ("ID64", [64, 64]), ("ID128", [128, 128])]:
        cdr[name] = nc.dram_tensor(name, shape, BF, kind="ExternalInput").ap()
    yp = nc.dram_tensor("yp", [B, C, H, W], F16, kind="ExternalOutput").ap()

    with tile.TileContext(nc) as tc, ExitStack() as ctx:
        # -------- constants to SBUF --------
        cp = ctx.enter_context(tc.tile_pool(name="consts", bufs=1))
        fh_sb, cwa_sb, cwb_sb = [], [], []
        for hb in range(2):
            t = cp.tile([128, 128], BF, tag=f"fh{hb}")
            nc.sync.dma_start(t[:], cdr["FH"][hb * 128:(hb + 1) * 128, :])
            fh_sb.append(t)
            ta = cp.tile([128, 64], BF, tag=f"cwa{hb}")
            nc.sync.dma_start(ta[:], cdr["CWA"][hb * 128:(hb + 1) * 128, :])
            cwa_sb.append(ta)
            tb = cp.tile([128, 64], BF, tag=f"cwb{hb}")
            nc.sync.dma_start(tb[:], cdr["CWB"][hb * 128:(hb + 1) * 128, :])
            cwb_sb.append(tb)
        eh_sb = cp.tile([128, 512], BF, tag="eh")
        nc.sync.dma_start(eh_sb[:], cdr["EH2"][:])
        ew_sb = cp.tile([128, 512], BF, tag="ew")
        nc.sync.dma_start(ew_sb[:], cdr["EW3"][:])
        id32 = cp.tile([32, 32], BF, tag="id32")
        nc.sync.dma_start(id32[:], cdr["ID32"][:])
        id64 = cp.tile([64, 64], BF, tag="id64")
        nc.sync.dma_start(id64[:], cdr["ID64"][:])
        id128 = cp.tile([128, 128], BF, tag="id128")
        nc.sync.dma_start(id128[:], cdr["ID128"][:])

        # -------- persistent intermediate tensors --------
        big = ctx.enter_context(tc.tile_pool(name="big", bufs=1))
        # Fbig cols: b*4096 + rF*2048 + ky*64 + rW*32 + i ; rows kx64
        fbig = big.tile([64, 16384], BF, tag="fbig")
        # FT cols: kx*256 + ky*8 + b*2 + rF ; rows (rW2, i32)
        ft = big.tile([64, 16384], BF, tag="ft")
        # P4 cols: ky*512 + b*128 + kx*2 + rF ; rows o32
        p4 = big.tile([32, 16384], BF, tag="p4")
        # TD cols: ky*128 + b*32 + o ; rows (kx64, rF2) interleaved kx*2+rF
        td = big.tile([128, 4096], BF, tag="td")
        # U cols: hh*128 + hp*64 + ky*2 + rU ; rows (b4, o32)
        u_sb = big.tile([128, 16384], BF, tag="u")

        # ================= Phase 1: stages A + B =================
        with ExitStack() as p1:
            xpool = p1.enter_context(tc.tile_pool(name="x", bufs=3))
            ztpool = p1.enter_context(tc.tile_pool(name="zt", bufs=4))
            psa = p1.enter_context(tc.tile_pool(name="psa", bufs=2, space="PSUM"))
            psb = p1.enter_context(tc.tile_pool(name="psb", bufs=2, space="PSUM"))
            for b in range(B):
                for i in range(C):
                    xt = xpool.tile([128, 2, 256], BF, tag="xt")
                    nc.sync.dma_start(
                        xt[:], xp[b, i].rearrange("(hb hp) w -> hp hb w", hb=2))
                    pa = psa.tile([128, 256], FP, tag="pa")
                    for ws in range(2):
                        for hb in range(2):
                            nc.tensor.matmul(
                                pa[:, ws * 128:ws * 128 + 128],
                                xt[:, hb, ws * 128: ws * 128 + 128],
                                fh_sb[hb][:],
                                start=(hb == 0), stop=(hb == 1))
                    zt = ztpool.tile([128, 256], BF, tag="zt")
                    nc.vector.tensor_copy(zt[:], pa[:])
                    pb = psb.tile([64, 64], FP, tag="pb")
                    for ws in range(2):
                        nc.tensor.matmul(pb[:], zt[:, ws * 128:ws * 128 + 64],
                                         cwa_sb[ws][:],
                                         start=(ws == 0), stop=False)
                        nc.tensor.matmul(pb[:], zt[:, ws * 128 + 64:ws * 128 + 128],
                                         cwb_sb[ws][:],
                                         start=False, stop=(ws == 1))
                    # Fbig col = b*4096 + rF*2048 + ky*64 + rW*32 + i
                    fb6 = fbig[:].rearrange(
                        "p (b rf ky rw i) -> p b rf ky rw i",
                        b=B, rf=2, ky=M2, rw=2, i=C)
                    # rW=0 rows of FT: [Fre | Fim]
                    nc.vector.tensor_copy(
                        fb6[:, b, :, :, 0, i],
                        pb[:].rearrange("p (rf ky) -> p rf ky", rf=2))
                    # rW=1 rows of FT: [-Fim | Fre]
                    nc.scalar.mul(fb6[:, b, 0, :, 1, i], pb[:, 32:64], -1.0)
                    nc.scalar.copy(fb6[:, b, 1, :, 1, i], pb[:, 0:32])

        # ================= Phase T1: F -> FT transpose =================
        ft5 = ft[:].rearrange("p (kx ky b rf) -> p kx ky rf b",
                              kx=KX, ky=M2, b=B, rf=2)
        with ExitStack() as pt1:
            pst = pt1.enter_context(tc.tile_pool(name="pst", bufs=4, space="PSUM"))
            for b in range(B):
                for rf in range(2):
                    for ky in range(M2):
                        c0 = b * 4096 + rf * 2048 + ky * 64
                        pt = pst.tile([64, 64], BF, tag="pt")
                        nc.tensor.transpose(pt[:], fbig[:, c0:c0 + 64], id64[:])
                        nc.vector.tensor_copy(ft5[:, :, ky, rf, b], pt[:])

        # ================= Phase MIX =================
        with ExitStack() as pm:
            wpool = pm.enter_context(tc.tile_pool(name="w", bufs=2))
            psm = pm.enter_context(tc.tile_pool(name="psm", bufs=4, space="PSUM"))
            for q in range(16):  # kx quad
                wt = wpool.tile([64, 4096], BF, tag="wt")  # cols o*128+kxs*32+ky
                if q < 8:
                    sre, sim, kxo = w1re, w1im, q * 4
                else:
                    sre, sim, kxo = w4re, w4im, (q - 8) * 4
                nc.sync.dma_start(
                    wt[0:32, :], sre[:, :, kxo:kxo + 4, :])
                nc.sync.dma_start(
                    wt[32:64, :], sim[:, :, kxo:kxo + 4, :])
                wt4 = wt[:].rearrange("p (o kxs ky) -> p o kxs ky", o=C, kxs=4)
                p45 = p4[:].rearrange("p (ky b kx rf) -> p ky b rf kx",
                                      ky=M2, b=B, kx=KX)
                for kxs in range(4):
                    kx = q * 4 + kxs
                    pm_t = psm.tile([32, 256], FP, tag="pmix")
                    for ky in range(M2):
                        nc.tensor.matmul(
                            pm_t[:, ky * 8:ky * 8 + 8],
                            wt4[:, :, kxs, ky],               # [64, 32]
                            ft[:, kx * 256 + ky * 8:kx * 256 + ky * 8 + 8],
                            start=True, stop=True)
                    # psum cols (ky, b, rF) -> P4 col ky*512 + b*128 + kx*2 + rF
                    if kx % 2 == 0:
                        nc.vector.tensor_copy(p45[:, :, :, :, kx], pm_t[:])
                    else:
                        nc.scalar.copy(p45[:, :, :, :, kx], pm_t[:])

        # ================= Phase T2 + D =================
        with ExitStack() as pt2:
            pst2 = pt2.enter_context(tc.tile_pool(name="pst2", bufs=4, space="PSUM"))
            for ky in range(M2):
                for b in range(B):
                    c0 = ky * 512 + b * 128
                    pt = pst2.tile([128, 32], BF, tag="pt2")
                    nc.tensor.transpose(pt[:], p4[:, c0:c0 + 128], id32[:])
                    if b % 2 == 0:
                        nc.vector.tensor_copy(
                            td[:, ky * 128 + b * 32:ky * 128 + b * 32 + 32],
                            pt[:])
                    else:
                        nc.scalar.copy(
                            td[:, ky * 128 + b * 32:ky * 128 + b * 32 + 32],
                            pt[:])
        with ExitStack() as pd_s:
            psd = pd_s.enter_context(tc.tile_pool(name="psd", bufs=3, space="PSUM"))
            # U cols: hh*128 + hp*64 + ky*2 + rU ; psum cols (hh, hp, rU)
            u4 = u_sb[:].rearrange("p (hh hp ky ru) -> p hh hp ru ky",
                                   hh=128, hp=2, ky=M2)
            for ky in range(M2):
                pd = psd.tile([128, 512], FP, tag="pd")
                nc.tensor.matmul(pd[:], td[:, ky * 128:(ky + 1) * 128],
                                 eh_sb[:], start=True, stop=True)
                if ky % 2 == 0:
                    nc.vector.tensor_copy(u4[:, :, :, :, ky], pd[:])
                else:
                    nc.scalar.copy(u4[:, :, :, :, ky], pd[:])

        # ================= Phase T3 + E + output =================
        yb = yp.rearrange("b o h w -> (b o) h w")
        with ExitStack() as pe_s:
            pst3 = pe_s.enter_context(tc.tile_pool(name="pst3", bufs=3, space="PSUM"))
            utp = pe_s.enter_context(tc.tile_pool(name="ut", bufs=3))
            pse = pe_s.enter_context(tc.tile_pool(name="pse", bufs=4, space="PSUM"))
            ostp = pe_s.enter_context(tc.tile_pool(name="ost", bufs=3))
            ost = None
            for hh in range(128):
                pt = pst3.tile([128, 128], BF, tag="pt3")
                nc.tensor.transpose(pt[:], u_sb[:, hh * 128:(hh + 1) * 128],
                                    id128[:])
                ut = utp.tile([128, 128], BF, tag="ut")
                if hh % 2 == 0:
                    nc.vector.tensor_copy(ut[:], pt[:])
                else:
                    nc.scalar.copy(ut[:], pt[:])
                if hh % 4 == 0:
                    ost = ostp.tile([128, 2048], F16, tag="ost")
                # ut rows (hp, ky, rU); EW3 block-diagonal in hp
                # out pe cols (hp, w) = rows h = 2*hh + hp
                pe = pse.tile([128, 512], FP, tag="pe")
                nc.tensor.matmul(pe[:], ut[:], ew_sb[:],
                                 start=True, stop=True)
                if hh % 2 == 0:
                    nc.scalar.copy(
                        ost[:, (hh % 4) * 512:(hh % 4) * 512 + 512], pe[:])
                else:
                    nc.vector.tensor_copy(
                        ost[:, (hh % 4) * 512:(hh % 4) * 512 + 512], pe[:])
                if hh % 4 == 3:
                    nc.sync.dma_start(
                        yb[:, (hh - 3) * 2:(hh + 1) * 2, :], ost[:])

    nc.compile()
    return nc


_NC = None


def kernel(x, w1_re, w1_im, w4_re, w4_im):
    global _NC
    if _NC is None:
        _NC = _build()
    consts = _constants()
    in_maps = []
    for p in range(P):
        m = {
            "xp": np.ascontiguousarray(x[p]).astype(BFNP),
            "w1re": np.ascontiguousarray(w1_re[:, :, p]).astype(BFNP),
            "w1im": np.ascontiguousarray(w1_im[:, :, p]).astype(BFNP),
            "w4re": np.ascontiguousarray(w4_re[:, :, p]).astype(BFNP),
            "w4im": np.ascontiguousarray(w4_im[:, :, p]).astype(BFNP),
        }
        m.update(consts)
        in_maps.append(m)
    res = run_bass_kernel_spmd(_NC, in_maps, core_ids=list(range(P)))
    return np.stack([res.results[p]["yp"] for p in range(P)],
                    axis=0).astype(np.float32)


if __name__ == "__main__":
    rng = np.random.default_rng(0)
    x = rng.standard_normal((P, B, C, H, W)).astype(np.float32)
    wshape = (C, C, P, M1, M2)
    ws = [(rng.random(wshape, np.float32) / (C * C)).astype(np.float32)
          for _ in range(4)]
    out = kernel(x, *ws)
    print("out", out.shape, out.dtype, float(np.abs(out).max()))



# revision 31
# speedup vs baseline: 1.3866x; 1.2389x over previous
"""Trainium2 Bass kernel for nn_GNO2d (spectral conv, method-25 branch).

Sharded over pipes P=8, one pipe per NeuronCore. Per pipe the computation is a
truncated 2-D rFFT -> per-mode complex channel mixing -> inverse rFFT,
implemented entirely as TensorEngine matmuls against small DFT constant
matrices (bf16 operands, fp32 PSUM accumulation):

  A: Z[kx,w]   = sum_h x[h,w] e^{-i th_kx h}          (64 retained kx rows)
  B: F[kx,ky]  = sum_w Z[kx,w] e^{-i ph_ky w}         (32 retained ky cols)
  T1: PE-transpose F from [kx,..] to [(rW,i),..] layout
  MIX: f[o,..] = sum_i W[i,o] F[i,..]  (complex, via K=(re/im,i)=64 matmuls)
  T2: PE-transpose f to [(kx,rF),..] layout
  D: U[.,h]    = sum_{kx,rF} f e^{+i th h}            (complex combine folded
  T3: PE-transpose U to [(hp,ky,rU),..] layout          into constant matrices)
  E: y[.,w]    = Re sum_{ky,rU} c_ky U e^{+i ph w} / (H W)
"""

import numpy as np
import ml_dtypes
from contextlib import ExitStack

import concourse.bass as bass
import concourse.tile as tile
import concourse.mybir as mybir
from concourse import bacc
from concourse.bass_utils import run_bass_kernel_spmd

P, B, C, H, W = 8, 4, 32, 256, 256
M1, M2 = 32, 32
KX = 2 * M1  # 64 retained kx rows
FP = mybir.dt.float32
BF = mybir.dt.bfloat16
BFNP = ml_dtypes.bfloat16


def _constants():
    """Host-side DFT constant matrices, bf16."""
    freqs = np.concatenate([np.arange(M1), np.arange(H - M1, H)])  # kx freqs
    th = 2 * np.pi * np.outer(np.arange(H), freqs) / H             # [H, KX]
    phi = 2 * np.pi * np.outer(np.arange(W), np.arange(M2)) / W    # [W, M2]

    fh = np.concatenate([np.cos(th), -np.sin(th)], axis=1)         # [256, 128]
    cwa = np.concatenate([np.cos(phi), -np.sin(phi)], axis=1)      # [256, 64]
    cwb = np.concatenate([np.sin(phi), np.cos(phi)], axis=1)       # [256, 64]

    # EH2 rows (kx*2 + rF), cols (hh*4 + hp*2 + rU)  [h = hh*2 + hp]
    eh = np.zeros((128, 512), np.float32)
    c, s = np.cos(th.T), np.sin(th.T)                              # [KX, H]
    eh[0::2, 0:256] = c      # (kx,re)->(re,h): +cos
    eh[0::2, 256:512] = s    # (kx,re)->(im,h): +sin
    eh[1::2, 0:256] = -s     # (kx,im)->(re,h): -sin
    eh[1::2, 256:512] = c    # (kx,im)->(im,h): +cos
    # reorder cols from (rU, hh, hp) to (hh, hp, rU)
    eh = np.ascontiguousarray(
        eh.reshape(128, 2, 128, 2).transpose(0, 2, 3, 1).reshape(128, 512))

    # EW3 rows (hp*64 + ky*2 + rU), cols (hp'*256 + w), block-diagonal in hp
    cky = np.where(np.arange(M2) == 0, 1.0, 2.0)[:, None]
    ewc = cky * np.cos(phi.T) / (H * W)                            # [M2, W]
    ews = cky * np.sin(phi.T) / (H * W)
    ew = np.zeros((128, 512), np.float32)
    for hp in range(2):
        ew[hp * 64:hp * 64 + 64:2, hp * 256:hp * 256 + 256] = ewc
        ew[hp * 64 + 1:hp * 64 + 64:2, hp * 256:hp * 256 + 256] = -ews

    consts = {
        "FH": fh, "CWA": cwa, "CWB": cwb, "EH2": eh, "EW3": ew,
        "ID32": np.eye(32, dtype=np.float32),
        "ID64": np.eye(64, dtype=np.float32),
        "ID128": np.eye(128, dtype=np.float32),
    }
    return {k: np.ascontiguousarray(v.astype(BFNP)) for k, v in consts.items()}


F16 = mybir.dt.float16


def _build():
    nc = bacc.Bacc("TRN2", target_bir_lowering=False, debug=False, num_devices=P)
    xp = nc.dram_tensor("xp", [B, C, H, W], BF, kind="ExternalInput").ap()
    # host-packed MIX weights: [q, kpar*64 + rw*32 + i, kxs*512 + kyh*32 + o]
    wm = nc.dram_tensor("wm", [16, 128, 2048], BF, kind="ExternalInput").ap()
    cdr = {}
    for name, shape in [("FH", [256, 128]), ("CWA", [256, 64]), ("CWB", [256, 64]),
                        ("EH2", [128, 512]), ("EW3", [128, 512]),
                        ("ID32", [32, 32]), ("ID64", [64, 64]), ("ID128", [128, 128])]:
        cdr[name] = nc.dram_tensor(name, shape, BF, kind="ExternalInput").ap()
    yp = nc.dram_tensor("yp", [B, C, H, W], F16, kind="ExternalOutput").ap()

    with tile.TileContext(nc) as tc, ExitStack() as ctx:
        # -------- constants to SBUF --------
        cp = ctx.enter_context(tc.tile_pool(name="consts", bufs=1))
        fh_sb, cwa_sb, cwb_sb = [], [], []
        for hb in range(2):
            t = cp.tile([128, 128], BF, tag=f"fh{hb}")
            nc.sync.dma_start(t[:], cdr["FH"][hb * 128:(hb + 1) * 128, :])
            fh_sb.append(t)
            ta = cp.tile([128, 64], BF, tag=f"cwa{hb}")
            nc.sync.dma_start(ta[:], cdr["CWA"][hb * 128:(hb + 1) * 128, :])
            cwa_sb.append(ta)
            tb = cp.tile([128, 64], BF, tag=f"cwb{hb}")
            nc.sync.dma_start(tb[:], cdr["CWB"][hb * 128:(hb + 1) * 128, :])
            cwb_sb.append(tb)
        eh_sb = cp.tile([128, 512], BF, tag="eh")
        nc.sync.dma_start(eh_sb[:], cdr["EH2"][:])
        ew_sb = cp.tile([128, 512], BF, tag="ew")
        nc.sync.dma_start(ew_sb[:], cdr["EW3"][:])
        id32 = cp.tile([32, 32], BF, tag="id32")
        nc.sync.dma_start(id32[:], cdr["ID32"][:])
        id64 = cp.tile([64, 64], BF, tag="id64")
        nc.sync.dma_start(id64[:], cdr["ID64"][:])
        id128 = cp.tile([128, 128], BF, tag="id128")
        nc.sync.dma_start(id128[:], cdr["ID128"][:])

        # -------- persistent intermediate tensors --------
        big = ctx.enter_context(tc.tile_pool(name="big", bufs=1))
        # Fbig cols: b*4096 + rF*2048 + ky*64 + rW*32 + i ; rows kx64
        fbig = big.tile([64, 16384], BF, tag="fbig")
        # FT cols: kx*128 + kyh*8 + b*2 + rF ; rows (kpar2, rW2, i32)
        ft = big.tile([128, 8192], BF, tag="ft")
        # P4 cols: kyh*512 + b*128 + kx*2 + rF ; rows (kpar2, o32)
        p4 = big.tile([64, 8192], BF, tag="p4")
        # TD cols: kyh*256 + kpar*128 + b*32 + o ; rows (kx64, rF2)
        td = big.tile([128, 4096], BF, tag="td")
        # U cols: hh*128 + hp*64 + ky*2 + rU ; rows (b4, o32)
        u_sb = big.tile([128, 16384], BF, tag="u")

        # ================= Phase 1: stages A + B =================
        with ExitStack() as p1:
            xpool = p1.enter_context(tc.tile_pool(name="x", bufs=3))
            ztpool = p1.enter_context(tc.tile_pool(name="zt", bufs=4))
            psa = p1.enter_context(tc.tile_pool(name="psa", bufs=2, space="PSUM"))
            psb = p1.enter_context(tc.tile_pool(name="psb", bufs=2, space="PSUM"))
            for b in range(B):
                for i2 in range(C // 2):
                    pa = psa.tile([128, 512], FP, tag="pa")
                    for j in range(2):
                        i = i2 * 2 + j
                        xt = xpool.tile([128, 2, 256], BF, tag="xt")
                        nc.sync.dma_start(
                            xt[:],
                            xp[b, i].rearrange("(hb hp) w -> hp hb w", hb=2))
                        for ws in range(2):
                            for hb in range(2):
                                nc.tensor.matmul(
                                    pa[:, j * 256 + ws * 128:
                                       j * 256 + ws * 128 + 128],
                                    xt[:, hb, ws * 128: ws * 128 + 128],
                                    fh_sb[hb][:],
                                    start=(hb == 0), stop=(hb == 1))
                    zt = ztpool.tile([128, 512], BF, tag="zt")
                    nc.vector.tensor_copy(zt[:], pa[:])
                    for j in range(2):
                        i = i2 * 2 + j
                        pb = psb.tile([64, 64], FP, tag="pb")
                        for ws in range(2):
                            z0 = j * 256 + ws * 128
                            nc.tensor.matmul(pb[:], zt[:, z0:z0 + 64],
                                             cwa_sb[ws][:],
                                             start=(ws == 0), stop=False)
                            nc.tensor.matmul(pb[:], zt[:, z0 + 64:z0 + 128],
                                             cwb_sb[ws][:],
                                             start=False, stop=(ws == 1))
                        # Fbig col = b*4096 + rF*2048 + ky*64 + rW*32 + i
                        fb6 = fbig[:].rearrange(
                            "p (b rf ky rw i) -> p b rf ky rw i",
                            b=B, rf=2, ky=M2, rw=2, i=C)
                        # rW=0 rows of FT: [Fre | Fim]
                        nc.vector.tensor_copy(
                            fb6[:, b, :, :, 0, i],
                            pb[:].rearrange("p (rf ky) -> p rf ky", rf=2))
                        # rW=1 rows of FT: [-Fim | Fre]
                        nc.scalar.mul(fb6[:, b, 0, :, 1, i], pb[:, 32:64], -1.0)
                        nc.scalar.copy(fb6[:, b, 1, :, 1, i], pb[:, 0:32])

        # ================= Phase T1: F -> FT transpose =================
        # transpose 2-ky chunks [64, 128] -> [128 rows (kpar, rW, i), 64 kx]
        ft5 = ft[:].rearrange("p (kx kyh b rf) -> p kx kyh rf b",
                              kx=KX, kyh=M2 // 2, b=B, rf=2)
        with ExitStack() as pt1:
            pst = pt1.enter_context(tc.tile_pool(name="pst", bufs=4, space="PSUM"))
            for b in range(B):
                for rf in range(2):
                    for kyh in range(M2 // 2):
                        c0 = b * 4096 + rf * 2048 + kyh * 128
                        pt = pst.tile([128, 64], BF, tag="pt")
                        nc.tensor.transpose(pt[:], fbig[:, c0:c0 + 128], id64[:])
                        if kyh % 2 == 0:
                            nc.vector.tensor_copy(ft5[:, :, kyh, rf, b], pt[:])
                        else:
                            nc.scalar.copy(ft5[:, :, kyh, rf, b], pt[:])

        # ================= Phase MIX =================
        # per mode (kx, ky = 2*kyh + kpar): stationary [64, 32] at partition
        # half kpar*64, rhs ft rows kpar*64.. -> 2 modes run on disjoint PE
        # quadrants concurrently (tile_position auto from base partitions)
        p45 = p4[:].rearrange("p (kyh b kx rf) -> p kyh b rf kx",
                              kyh=M2 // 2, b=B, kx=KX)
        with ExitStack() as pm:
            wpool = pm.enter_context(tc.tile_pool(name="w", bufs=2))
            psm = pm.enter_context(tc.tile_pool(name="psm", bufs=4, space="PSUM"))
            for q in range(16):  # kx quad
                wt = wpool.tile([128, 2048], BF, tag="wt")
                nc.sync.dma_start(wt[:], wm[q])
                for kxs in range(4):
                    kx = q * 4 + kxs
                    pm_t = psm.tile([64, 128], FP, tag="pmix")
                    for kyh in range(M2 // 2):
                        for kpar in range(2):
                            nc.tensor.matmul(
                                pm_t[kpar * 32:kpar * 32 + 32,
                                     kyh * 8:kyh * 8 + 8],
                                wt[kpar * 64:kpar * 64 + 64,
                                   kxs * 512 + kyh * 32:
                                   kxs * 512 + kyh * 32 + 32],
                                ft[kpar * 64:kpar * 64 + 64,
                                   kx * 128 + kyh * 8:kx * 128 + kyh * 8 + 8],
                                start=True, stop=True)
                    # psum [ (kpar,o), (kyh, b, rF) ] -> P4 cols
                    if kx % 2 == 0:
                        nc.vector.tensor_copy(p45[:, :, :, :, kx], pm_t[:])
                    else:
                        nc.scalar.copy(p45[:, :, :, :, kx], pm_t[:])

        # ================= Phase T2 + D =================
        # T2: transpose [64 (kpar,o), 128 (kx,rF)] chunks of p4
        #   -> td cols (kyh, kpar, b, o), rows (kx, rF)
        td4 = td[:].rearrange("p (kyh kpar b o) -> p kyh b kpar o",
                              kyh=M2 // 2, kpar=2, b=B)
        with ExitStack() as pt2:
            pst2 = pt2.enter_context(tc.tile_pool(name="pst2", bufs=4, space="PSUM"))
            for kyh in range(M2 // 2):
                for b in range(B):
                    c0 = kyh * 512 + b * 128
                    pt = pst2.tile([128, 64], BF, tag="pt2")
                    nc.tensor.transpose(pt[:], p4[:, c0:c0 + 128], id64[:])
                    if b % 2 == 0:
                        nc.vector.tensor_copy(td4[:, kyh, b], pt[:])
                    else:
                        nc.scalar.copy(td4[:, kyh, b], pt[:])
        with ExitStack() as pd_s:
            psd = pd_s.enter_context(tc.tile_pool(name="psd", bufs=3, space="PSUM"))
            # U cols: hh*128 + hp*64 + ky*2 + rU ; psum cols (hh, hp, rU)
            u4 = u_sb[:].rearrange("p (hh hp ky ru) -> p hh hp ru ky",
                                   hh=128, hp=2, ky=M2)
            for ky in range(M2):
                kyh, kpar = ky // 2, ky % 2
                pd = psd.tile([128, 512], FP, tag="pd")
                nc.tensor.matmul(
                    pd[:], td[:, kyh * 256 + kpar * 128:
                               kyh * 256 + kpar * 128 + 128],
                    eh_sb[:], start=True, stop=True)
                if ky % 2 == 0:
                    nc.vector.tensor_copy(u4[:, :, :, :, ky], pd[:])
                else:
                    nc.scalar.copy(u4[:, :, :, :, ky], pd[:])

        # ================= Phase T3 + E + output =================
        yb = yp.rearrange("b o h w -> (b o) h w")
        with ExitStack() as pe_s:
            pst3 = pe_s.enter_context(tc.tile_pool(name="pst3", bufs=3, space="PSUM"))
            utp = pe_s.enter_context(tc.tile_pool(name="ut", bufs=3))
            pse = pe_s.enter_context(tc.tile_pool(name="pse", bufs=4, space="PSUM"))
            ostp = pe_s.enter_context(tc.tile_pool(name="ost", bufs=3))
            ost = None
            for hh in range(128):
                pt = pst3.tile([128, 128], BF, tag="pt3")
                nc.tensor.transpose(pt[:], u_sb[:, hh * 128:(hh + 1) * 128],
                                    id128[:])
                ut = utp.tile([128, 128], BF, tag="ut")
                if hh % 2 == 0:
                    nc.vector.tensor_copy(ut[:], pt[:])
                else:
                    nc.scalar.copy(ut[:], pt[:])
                if hh % 4 == 0:
                    ost = ostp.tile([128, 2048], F16, tag="ost")
                # ut rows (hp, ky, rU); EW3 block-diagonal in hp
                # out pe cols (hp, w) = rows h = 2*hh + hp
                pe = pse.tile([128, 512], FP, tag="pe")
                nc.tensor.matmul(pe[:], ut[:], ew_sb[:],
                                 start=True, stop=True)
                if hh % 2 == 0:
                    nc.scalar.copy(
                        ost[:, (hh % 4) * 512:(hh % 4) * 512 + 512], pe[:])
                else:
                    nc.vector.tensor_copy(
                        ost[:, (hh % 4) * 512:(hh % 4) * 512 + 512], pe[:])
                if hh % 4 == 3:
                    nc.sync.dma_start(
                        yb[:, (hh - 3) * 2:(hh + 1) * 2, :], ost[:])

    nc.compile()
    return nc


_NC = None


def kernel(x, w1_re, w1_im, w4_re, w4_im):
    global _NC
    if _NC is None:
        _NC = _build()
    consts = _constants()
    in_maps = []
    for p in range(P):
        # pack MIX weights: [q, kpar*64 + rw*32 + i, kxs*512 + kyh*32 + o]
        wre = np.concatenate([w1_re[:, :, p], w4_re[:, :, p]], axis=2)
        wim = np.concatenate([w1_im[:, :, p], w4_im[:, :, p]], axis=2)
        wall = np.stack([wre, wim], axis=0)          # [rw, i, o, kx64, ky32]
        wall = wall.reshape(2, C, C, 16, 4, 16, 2)   # rw i o q kxs kyh kpar
        wall = wall.transpose(3, 6, 0, 1, 4, 5, 2)   # q kpar rw i kxs kyh o
        wmp = np.ascontiguousarray(wall.reshape(16, 128, 2048)).astype(BFNP)
        m = {
            "xp": np.ascontiguousarray(x[p]).astype(BFNP),
            "wm": wmp,
        }
        m.update(consts)
        in_maps.append(m)
    res = run_bass_kernel_spmd(_NC, in_maps, core_ids=list(range(P)))
    return np.stack([res.results[p]["yp"] for p in range(P)],
                    axis=0).astype(np.float32)


if __name__ == "__main__":
    rng = np.random.default_rng(0)
    x = rng.standard_normal((P, B, C, H, W)).astype(np.float32)
    wshape = (C, C, P, M1, M2)
    ws = [(rng.random(wshape, np.float32) / (C * C)).astype(np.float32)
          for _ in range(4)]
    out = kernel(x, *ws)
    print("out", out.shape, out.dtype, float(np.abs(out).max()))



# revision 43
# speedup vs baseline: 1.4016x; 1.0109x over previous
"""Trainium2 Bass kernel for nn_GNO2d (spectral conv, method-25 branch).

Sharded over pipes P=8, one pipe per NeuronCore. Per pipe the computation is a
truncated 2-D rFFT -> per-mode complex channel mixing -> inverse rFFT,
implemented entirely as TensorEngine matmuls against small DFT constant
matrices (bf16 operands, fp32 PSUM accumulation):

  A: Z[kx,w]   = sum_h x[h,w] e^{-i th_kx h}          (64 retained kx rows)
  B: F[kx,ky]  = sum_w Z[kx,w] e^{-i ph_ky w}         (32 retained ky cols)
  T1: PE-transpose F from [kx,..] to [(rW,i),..] layout
  MIX: f[o,..] = sum_i W[i,o] F[i,..]  (complex, via K=(re/im,i)=64 matmuls)
  T2: PE-transpose f to [(kx,rF),..] layout
  D: U[.,h]    = sum_{kx,rF} f e^{+i th h}            (complex combine folded
  T3: PE-transpose U to [(hp,ky,rU),..] layout          into constant matrices)
  E: y[.,w]    = Re sum_{ky,rU} c_ky U e^{+i ph w} / (H W)
"""

import numpy as np
import ml_dtypes
from contextlib import ExitStack

import concourse.bass as bass
import concourse.tile as tile
import concourse.mybir as mybir
from concourse import bacc
from concourse.bass_utils import run_bass_kernel_spmd

P, B, C, H, W = 8, 4, 32, 256, 256
M1, M2 = 32, 32
KX = 2 * M1  # 64 retained kx rows
FP = mybir.dt.float32
BF = mybir.dt.bfloat16
BFNP = ml_dtypes.bfloat16


def _constants():
    """Host-side DFT constant matrices, bf16."""
    freqs = np.concatenate([np.arange(M1), np.arange(H - M1, H)])  # kx freqs
    th = 2 * np.pi * np.outer(np.arange(H), freqs) / H             # [H, KX]
    phi = 2 * np.pi * np.outer(np.arange(W), np.arange(M2)) / W    # [W, M2]

    fh = np.concatenate([np.cos(th), -np.sin(th)], axis=1)         # [256, 128]
    cwa = np.concatenate([np.cos(phi), -np.sin(phi)], axis=1)      # [256, 64]
    cwb = np.concatenate([np.sin(phi), np.cos(phi)], axis=1)       # [256, 64]

    # EHRE/EHIM rows kx, cols (hh*4 + hp*2 + rU)  [h = hh*2 + hp]
    # U_ru = sum_kx Gre*(ru? sin:cos) + Gim*(ru? cos:-sin)
    c, s = np.cos(th.T), np.sin(th.T)                              # [KX, H]
    ehre = np.stack([c, s], axis=2).reshape(64, 512)       # cols (h, rU)
    ehim = np.stack([-s, c], axis=2).reshape(64, 512)
    # (h, rU) with h=(hh,hp) -> (hh, hp, rU) is already the natural order
    # since h*2+ru = hh*4 + hp*2 + ru

    # EW3 rows (hp*64 + ky*2 + rU), cols (hp'*256 + w), block-diagonal in hp
    cky = np.where(np.arange(M2) == 0, 1.0, 2.0)[:, None]
    ewc = cky * np.cos(phi.T) / (H * W)                            # [M2, W]
    ews = cky * np.sin(phi.T) / (H * W)
    ew = np.zeros((128, 512), np.float32)
    for hp in range(2):
        ew[hp * 64:hp * 64 + 64:2, hp * 256:hp * 256 + 256] = ewc
        ew[hp * 64 + 1:hp * 64 + 64:2, hp * 256:hp * 256 + 256] = -ews

    consts = {
        "FH": fh, "CWA": cwa, "CWB": cwb,
        "EHRE": ehre, "EHIM": ehim, "EW3": ew,
        "ID64": np.eye(64, dtype=np.float32),
        "ID128": np.eye(128, dtype=np.float32),
    }
    return {k: np.ascontiguousarray(v.astype(BFNP)) for k, v in consts.items()}


F16 = mybir.dt.float16


def _build():
    nc = bacc.Bacc("TRN2", target_bir_lowering=False, debug=False, num_devices=P)
    xp = nc.dram_tensor("xp", [B, C, H, W], BF, kind="ExternalInput").ap()
    # host-packed MIX embeddings:
    #   [q, kpar*64 + i*2 + c, kxs*1024 + kyh*64 + rf*32 + o]
    wm = nc.dram_tensor("wm", [16, 128, 4096], BF, kind="ExternalInput").ap()
    cdr = {}
    for name, shape in [("FH", [256, 128]), ("CWA", [256, 64]), ("CWB", [256, 64]),
                        ("EHRE", [64, 512]), ("EHIM", [64, 512]),
                        ("EW3", [128, 512]),
                        ("ID64", [64, 64]), ("ID128", [128, 128])]:
        cdr[name] = nc.dram_tensor(name, shape, BF, kind="ExternalInput").ap()
    yp = nc.dram_tensor("yp", [B, C, H, W], F16, kind="ExternalOutput").ap()

    with tile.TileContext(nc) as tc, ExitStack() as ctx:
        # -------- constants to SBUF --------
        cp = ctx.enter_context(tc.tile_pool(name="consts", bufs=1))
        fh_sb, cwa_sb, cwb_sb = [], [], []
        for hb in range(2):
            t = cp.tile([128, 128], BF, tag=f"fh{hb}")
            nc.sync.dma_start(t[:], cdr["FH"][hb * 128:(hb + 1) * 128, :])
            fh_sb.append(t)
            ta = cp.tile([128, 64], BF, tag=f"cwa{hb}")
            nc.sync.dma_start(ta[:], cdr["CWA"][hb * 128:(hb + 1) * 128, :])
            cwa_sb.append(ta)
            tb = cp.tile([128, 64], BF, tag=f"cwb{hb}")
            nc.sync.dma_start(tb[:], cdr["CWB"][hb * 128:(hb + 1) * 128, :])
            cwb_sb.append(tb)
        ehre_sb = cp.tile([64, 512], BF, tag="ehre")
        nc.sync.dma_start(ehre_sb[:], cdr["EHRE"][:])
        ehim_sb = cp.tile([64, 512], BF, tag="ehim")
        nc.sync.dma_start(ehim_sb[:], cdr["EHIM"][:])
        ew_sb = cp.tile([128, 512], BF, tag="ew")
        nc.sync.dma_start(ew_sb[:], cdr["EW3"][:])
        id64 = cp.tile([64, 64], BF, tag="id64")
        nc.sync.dma_start(id64[:], cdr["ID64"][:])
        id128 = cp.tile([128, 128], BF, tag="id128")
        nc.sync.dma_start(id128[:], cdr["ID128"][:])

        # -------- persistent intermediate tensors --------
        big = ctx.enter_context(tc.tile_pool(name="big", bufs=1))
        # Fbig cols: b*2048 + kyh*128 + kpar*64 + i*2 + c ; rows kx64
        fbig = big.tile([64, 8192], BF, tag="fbig")
        # FT cols: kx*64 + kyh*4 + b ; rows (kpar2, i32, c2)
        ft = big.tile([128, 4096], BF, tag="ft")
        # P4 cols: kyh*256 + b*64 + kx ; rows (kpar2, rF2, o32)
        p4 = big.tile([128, 4096], BF, tag="p4")
        # TD cols: kyh*512 + kpar*256 + rF*128 + b*32 + o ; rows kx64
        td = big.tile([64, 8192], BF, tag="td")
        # U cols: hh*128 + hp*64 + ky*2 + rU ; rows (b4, o32)
        u_sb = big.tile([128, 16384], BF, tag="u")

        # ================= Phase 1: stages A + B =================
        with ExitStack() as p1:
            xpool = p1.enter_context(tc.tile_pool(name="x", bufs=3))
            ztpool = p1.enter_context(tc.tile_pool(name="zt", bufs=4))
            psa = p1.enter_context(tc.tile_pool(name="psa", bufs=2, space="PSUM"))
            psb = p1.enter_context(tc.tile_pool(name="psb", bufs=2, space="PSUM"))
            for b in range(B):
                for i2 in range(C // 2):
                    pa = psa.tile([128, 512], FP, tag="pa")
                    for j in range(2):
                        i = i2 * 2 + j
                        xt = xpool.tile([128, 2, 256], BF, tag="xt")
                        nc.sync.dma_start(
                            xt[:],
                            xp[b, i].rearrange("(hb hp) w -> hp hb w", hb=2))
                        for ws in range(2):
                            for hb in range(2):
                                nc.tensor.matmul(
                                    pa[:, j * 256 + ws * 128:
                                       j * 256 + ws * 128 + 128],
                                    xt[:, hb, ws * 128: ws * 128 + 128],
                                    fh_sb[hb][:],
                                    start=(hb == 0), stop=(hb == 1))
                    zt = ztpool.tile([128, 512], BF, tag="zt")
                    if i2 % 2 == 0:
                        nc.vector.tensor_copy(zt[:], pa[:])
                    else:
                        nc.scalar.copy(zt[:], pa[:])
                    pb = psb.tile([64, 128], FP, tag="pb")
                    for j in range(2):
                        for ws in range(2):
                            z0 = j * 256 + ws * 128
                            nc.tensor.matmul(pb[:, j * 64:j * 64 + 64],
                                             zt[:, z0:z0 + 64],
                                             cwa_sb[ws][:],
                                             start=(ws == 0), stop=False)
                            nc.tensor.matmul(pb[:, j * 64:j * 64 + 64],
                                             zt[:, z0 + 64:z0 + 128],
                                             cwb_sb[ws][:],
                                             start=False, stop=(ws == 1))
                    # pb cols (j2, c2, ky32) -> Fbig (kyh, kpar, i=2*i2+j, c)
                    fb8 = fbig[:].rearrange(
                        "p (b kyh kpar i2 j c) -> p b i2 kyh kpar j c",
                        b=B, kyh=M2 // 2, kpar=2, i2=C // 2, j=2)
                    pb8 = pb[:].rearrange(
                        "p (j c kyh kpar) -> p kyh kpar j c",
                        j=2, c=2, kyh=M2 // 2)
                    if i2 % 2 == 1:
                        nc.vector.tensor_copy(fb8[:, b, i2], pb8)
                    else:
                        nc.scalar.copy(fb8[:, b, i2], pb8)

        # ================= Phase T1: F -> FT transpose =================
        # transpose chunks [64 kx, 128 (kpar, i, c)] -> [128, 64 kx]
        ft5 = ft[:].rearrange("p (kx kyh b) -> p kx kyh b",
                              kx=KX, kyh=M2 // 2, b=B)
        with ExitStack() as pt1:
            pst = pt1.enter_context(tc.tile_pool(name="pst", bufs=4, space="PSUM"))
            for b in range(B):
                for kyh in range(M2 // 2):
                    c0 = b * 2048 + kyh * 128
                    pt = pst.tile([128, 64], BF, tag="pt")
                    nc.tensor.transpose(pt[:], fbig[:, c0:c0 + 128], id64[:])
                    if kyh % 2 == 0:
                        nc.vector.tensor_copy(ft5[:, :, kyh, b], pt[:])
                    else:
                        nc.scalar.copy(ft5[:, :, kyh, b], pt[:])

        # ================= Phase MIX =================
        # per mode (kx, ky = 2*kyh + kpar): stationary embedding [64 (i,c),
        # 64 (rF,o)] at partition half kpar*64, rhs ft rows kpar*64.. ->
        # 2 modes run on disjoint PE quadrants concurrently
        p45 = p4[:].rearrange("p (kyh b kx) -> p kyh b kx",
                              kyh=M2 // 2, b=B, kx=KX)
        with ExitStack() as pm:
            wpool = pm.enter_context(tc.tile_pool(name="w", bufs=2))
            psm = pm.enter_context(tc.tile_pool(name="psm", bufs=4, space="PSUM"))
            for q in range(16):  # kx quad
                wt = wpool.tile([128, 4096], BF, tag="wt")
                nc.sync.dma_start(wt[:], wm[q])
                for kxs in range(4):
                    kx = q * 4 + kxs
                    pm_t = psm.tile([128, 64], FP, tag="pmix")
                    for kyh in range(M2 // 2):
                        for kpar in range(2):
                            nc.tensor.matmul(
                                pm_t[kpar * 64:kpar * 64 + 64,
                                     kyh * 4:kyh * 4 + 4],
                                wt[kpar * 64:kpar * 64 + 64,
                                   kxs * 1024 + kyh * 64:
                                   kxs * 1024 + kyh * 64 + 64],
                                ft[kpar * 64:kpar * 64 + 64,
                                   kx * 64 + kyh * 4:kx * 64 + kyh * 4 + 4],
                                start=True, stop=True)
                    # psum rows (kpar, rF, o), cols (kyh, b) -> P4 col
                    if kx % 2 == 0:
                        nc.vector.tensor_copy(p45[:, :, :, kx], pm_t[:])
                    else:
                        nc.scalar.copy(p45[:, :, :, kx], pm_t[:])

        # ================= Phase T2 + D =================
        # T2: transpose [128 (kpar,rF,o), 64 kx] chunks of p4 ->
        #   td rows kx, cols (kyh, kpar, rF, b, o)
        td4 = td[:].rearrange("p (kyh kpar rf b o) -> p kyh b kpar rf o",
                              kyh=M2 // 2, kpar=2, rf=2, b=B)
        with ExitStack() as pt2:
            pst2 = pt2.enter_context(tc.tile_pool(name="pst2", bufs=4, space="PSUM"))
            for kyh in range(M2 // 2):
                for b in range(B):
                    c0 = kyh * 256 + b * 64
                    pt = pst2.tile([64, 128], BF, tag="pt2")
                    nc.tensor.transpose(pt[:], p4[:, c0:c0 + 64], id128[:])
                    if b % 2 == 0:
                        nc.vector.tensor_copy(td4[:, kyh, b], pt[:])
                    else:
                        nc.scalar.copy(td4[:, kyh, b], pt[:])
        with ExitStack() as pd_s:
            psd = pd_s.enter_context(tc.tile_pool(name="psd", bufs=3, space="PSUM"))
            # U cols: hh*128 + hp*64 + ky*2 + rU ; psum cols (hh, hp, rU)
            u4 = u_sb[:].rearrange("p (hh hp ky ru) -> p hh hp ru ky",
                                   hh=128, hp=2, ky=M2)
            for ky in range(M2):
                kyh, kpar = ky // 2, ky % 2
                pd = psd.tile([128, 512], FP, tag="pd")
                g0 = kyh * 512 + kpar * 256
                nc.tensor.matmul(pd[:], td[:, g0:g0 + 128],
                                 ehre_sb[:], start=True, stop=False)
                nc.tensor.matmul(pd[:], td[:, g0 + 128:g0 + 256],
                                 ehim_sb[:], start=False, stop=True)
                if ky % 2 == 0:
                    nc.vector.tensor_copy(u4[:, :, :, :, ky], pd[:])
                else:
                    nc.scalar.copy(u4[:, :, :, :, ky], pd[:])

        # ================= Phase T3 + E + output =================
        yb = yp.rearrange("b o h w -> (b o) h w")
        with ExitStack() as pe_s:
            pst3 = pe_s.enter_context(tc.tile_pool(name="pst3", bufs=3, space="PSUM"))
            utp = pe_s.enter_context(tc.tile_pool(name="ut", bufs=3))
            pse = pe_s.enter_context(tc.tile_pool(name="pse", bufs=4, space="PSUM"))
            ostp = pe_s.enter_context(tc.tile_pool(name="ost", bufs=3))
            ost = None
            for hh in range(128):
                pt = pst3.tile([128, 128], BF, tag="pt3")
                nc.tensor.transpose(pt[:], u_sb[:, hh * 128:(hh + 1) * 128],
                                    id128[:])
                ut = utp.tile([128, 128], BF, tag="ut")
                if hh % 2 == 0:
                    nc.vector.tensor_copy(ut[:], pt[:])
                else:
                    nc.scalar.copy(ut[:], pt[:])
                if hh % 4 == 0:
                    ost = ostp.tile([128, 2048], F16, tag="ost")
                # ut rows (hp, ky, rU); EW3 block-diagonal in hp
                # out pe cols (hp, w) = rows h = 2*hh + hp
                pe = pse.tile([128, 512], FP, tag="pe")
                nc.tensor.matmul(pe[:], ut[:], ew_sb[:],
                                 start=True, stop=True)
                if hh % 2 == 0:
                    nc.scalar.copy(
                        ost[:, (hh % 4) * 512:(hh % 4) * 512 + 512], pe[:])
                else:
                    nc.vector.tensor_copy(
                        ost[:, (hh % 4) * 512:(hh % 4) * 512 + 512], pe[:])
                if hh % 4 == 3:
                    nc.sync.dma_start(
                        yb[:, (hh - 3) * 2:(hh + 1) * 2, :], ost[:])

    nc.compile()
    return nc


_NC = None


def kernel(x, w1_re, w1_im, w4_re, w4_im):
    global _NC
    if _NC is None:
        _NC = _build()
    consts = _constants()
    in_maps = []
    for p in range(P):
        # pack MIX embeddings:
        #   [q, kpar*64 + i*2 + c, kxs*1024 + kyh*64 + rf*32 + o]
        # rows c=0 (Fre): [Wre | Wim]; rows c=1 (Fim): [-Wim | Wre]
        wre = np.concatenate([w1_re[:, :, p], w4_re[:, :, p]], axis=2)
        wim = np.concatenate([w1_im[:, :, p], w4_im[:, :, p]], axis=2)
        emb = np.empty((2, 2, C, C, KX, M2), np.float32)  # [c, rf, i, o, ...]
        emb[0, 0] = wre
        emb[0, 1] = wim
        emb[1, 0] = -wim
        emb[1, 1] = wre
        emb = emb.reshape(2, 2, C, C, 16, 4, 16, 2)  # c rf i o q kxs kyh kpar
        emb = emb.transpose(4, 7, 2, 0, 5, 6, 1, 3)  # q kpar i c kxs kyh rf o
        wmp = np.ascontiguousarray(emb.reshape(16, 128, 4096)).astype(BFNP)
        m = {
            "xp": np.ascontiguousarray(x[p]).astype(BFNP),
            "wm": wmp,
        }
        m.update(consts)
        in_maps.append(m)
    res = run_bass_kernel_spmd(_NC, in_maps, core_ids=list(range(P)))
    return np.stack([res.results[p]["yp"] for p in range(P)],
                    axis=0).astype(np.float32)


if __name__ == "__main__":
    rng = np.random.default_rng(0)
    x = rng.standard_normal((P, B, C, H, W)).astype(np.float32)
    wshape = (C, C, P, M1, M2)
    ws = [(rng.random(wshape, np.float32) / (C * C)).astype(np.float32)
          for _ in range(4)]
    out = kernel(x, *ws)
    print("out", out.shape, out.dtype, float(np.abs(out).max()))



# revision 44
# speedup vs baseline: 1.7232x; 1.2294x over previous
"""Trainium2 Bass kernel for nn_GNO2d (spectral conv, method-25 branch).

Sharded over pipes P=8, one pipe per NeuronCore. Per pipe the computation is a
truncated 2-D rFFT -> per-mode complex channel mixing -> inverse rFFT,
implemented entirely as TensorEngine matmuls against small DFT constant
matrices (bf16 operands, fp32 PSUM accumulation):

  A: Z[kx,w]   = sum_h x[h,w] e^{-i th_kx h}          (64 retained kx rows)
  B: F[kx,ky]  = sum_w Z[kx,w] e^{-i ph_ky w}         (32 retained ky cols)
  T1: PE-transpose F from [kx,..] to [(rW,i),..] layout
  MIX: f[o,..] = sum_i W[i,o] F[i,..]  (complex, via K=(re/im,i)=64 matmuls)
  T2: PE-transpose f to [(kx,rF),..] layout
  D: U[.,h]    = sum_{kx,rF} f e^{+i th h}            (complex combine folded
  T3: PE-transpose U to [(hp,ky,rU),..] layout          into constant matrices)
  E: y[.,w]    = Re sum_{ky,rU} c_ky U e^{+i ph w} / (H W)
"""

import numpy as np
import ml_dtypes
from contextlib import ExitStack

import concourse.bass as bass
import concourse.tile as tile
import concourse.mybir as mybir
from concourse import bacc
from concourse.bass_utils import run_bass_kernel_spmd

P, B, C, H, W = 8, 4, 32, 256, 256
M1, M2 = 32, 32
KX = 2 * M1  # 64 retained kx rows
FP = mybir.dt.float32
BF = mybir.dt.bfloat16
BFNP = ml_dtypes.bfloat16


def _constants():
    """Host-side DFT constant matrices, bf16."""
    freqs = np.concatenate([np.arange(M1), np.arange(H - M1, H)])  # kx freqs
    th = 2 * np.pi * np.outer(np.arange(H), freqs) / H             # [H, KX]
    phi = 2 * np.pi * np.outer(np.arange(W), np.arange(M2)) / W    # [W, M2]

    fh = np.concatenate([np.cos(th), -np.sin(th)], axis=1)         # [256, 128]
    cwa = np.concatenate([np.cos(phi), -np.sin(phi)], axis=1)      # [256, 64]
    cwb = np.concatenate([np.sin(phi), np.cos(phi)], axis=1)       # [256, 64]

    # EHRE/EHIM rows kx, cols (hh*4 + hp*2 + rU)  [h = hh*2 + hp]
    # U_ru = sum_kx Gre*(ru? sin:cos) + Gim*(ru? cos:-sin)
    c, s = np.cos(th.T), np.sin(th.T)                              # [KX, H]
    ehre = np.stack([c, s], axis=2).reshape(64, 512)       # cols (h, rU)
    ehim = np.stack([-s, c], axis=2).reshape(64, 512)
    # (h, rU) with h=(hh,hp) -> (hh, hp, rU) is already the natural order
    # since h*2+ru = hh*4 + hp*2 + ru

    # EW3 rows (hp*64 + ky*2 + rU), cols (hp'*256 + w), block-diagonal in hp
    cky = np.where(np.arange(M2) == 0, 1.0, 2.0)[:, None]
    ewc = cky * np.cos(phi.T) / (H * W)                            # [M2, W]
    ews = cky * np.sin(phi.T) / (H * W)
    ew = np.zeros((128, 512), np.float32)
    for hp in range(2):
        ew[hp * 64:hp * 64 + 64:2, hp * 256:hp * 256 + 256] = ewc
        ew[hp * 64 + 1:hp * 64 + 64:2, hp * 256:hp * 256 + 256] = -ews

    consts = {
        "FH": fh, "CWA": cwa, "CWB": cwb,
        "EHRE": ehre, "EHIM": ehim, "EW3": ew,
        "ID64": np.eye(64, dtype=np.float32),
        "ID128": np.eye(128, dtype=np.float32),
    }
    return {k: np.ascontiguousarray(v.astype(BFNP)) for k, v in consts.items()}


F16 = mybir.dt.float16


def _build():
    nc = bacc.Bacc("TRN2", target_bir_lowering=False, debug=False, num_devices=P)
    xp = nc.dram_tensor("xp", [B, C, H, W], BF, kind="ExternalInput").ap()
    # host-packed MIX embeddings:
    #   [q, kpar*64 + i*2 + c, kxs*1024 + kyh*64 + rf*32 + o]
    wm = nc.dram_tensor("wm", [16, 128, 4096], BF, kind="ExternalInput").ap()
    cdr = {}
    for name, shape in [("FH", [256, 128]), ("CWA", [256, 64]), ("CWB", [256, 64]),
                        ("EHRE", [64, 512]), ("EHIM", [64, 512]),
                        ("EW3", [128, 512]),
                        ("ID64", [64, 64]), ("ID128", [128, 128])]:
        cdr[name] = nc.dram_tensor(name, shape, BF, kind="ExternalInput").ap()
    yp = nc.dram_tensor("yp", [B, C, H, W], F16, kind="ExternalOutput").ap()

    with tile.TileContext(nc) as tc, ExitStack() as ctx:
        # -------- constants to SBUF --------
        cp = ctx.enter_context(tc.tile_pool(name="consts", bufs=1))
        fh_sb, cwa_sb, cwb_sb = [], [], []
        for hb in range(2):
            t = cp.tile([128, 128], BF, tag=f"fh{hb}")
            nc.sync.dma_start(t[:], cdr["FH"][hb * 128:(hb + 1) * 128, :])
            fh_sb.append(t)
            ta = cp.tile([128, 64], BF, tag=f"cwa{hb}")
            nc.sync.dma_start(ta[:], cdr["CWA"][hb * 128:(hb + 1) * 128, :])
            cwa_sb.append(ta)
            tb = cp.tile([128, 64], BF, tag=f"cwb{hb}")
            nc.sync.dma_start(tb[:], cdr["CWB"][hb * 128:(hb + 1) * 128, :])
            cwb_sb.append(tb)
        ehre_sb = cp.tile([64, 512], BF, tag="ehre")
        nc.sync.dma_start(ehre_sb[:], cdr["EHRE"][:])
        ehim_sb = cp.tile([64, 512], BF, tag="ehim")
        nc.sync.dma_start(ehim_sb[:], cdr["EHIM"][:])
        ew_sb = cp.tile([128, 512], BF, tag="ew")
        nc.sync.dma_start(ew_sb[:], cdr["EW3"][:])
        id64 = cp.tile([64, 64], BF, tag="id64")
        nc.sync.dma_start(id64[:], cdr["ID64"][:])
        id128 = cp.tile([128, 128], BF, tag="id128")
        nc.sync.dma_start(id128[:], cdr["ID128"][:])

        # -------- persistent intermediate tensors --------
        big = ctx.enter_context(tc.tile_pool(name="big", bufs=1))
        # Fbig cols: b*2048 + kyh*128 + kpar*64 + i*2 + c ; rows kx64
        fbig = big.tile([64, 8192], BF, tag="fbig")
        # FT cols: kx*64 + kyh*4 + b ; rows (kpar2, i32, c2)
        ft = big.tile([128, 4096], BF, tag="ft")
        # P4 cols: kyh*256 + b*64 + kx ; rows (kpar2, rF2, o32)
        p4 = big.tile([128, 4096], BF, tag="p4")
        # TD cols: kyh*512 + kpar*256 + rF*128 + b*32 + o ; rows kx64
        td = big.tile([64, 8192], BF, tag="td")
        # U cols: hh*128 + hp*64 + ky*2 + rU ; rows (b4, o32)
        u_sb = big.tile([128, 16384], BF, tag="u")

        # ================= Phase 1: stages A + B =================
        with ExitStack() as p1:
            xpool = p1.enter_context(tc.tile_pool(name="x", bufs=2))
            ztpool = p1.enter_context(tc.tile_pool(name="zt", bufs=6))
            psa = p1.enter_context(tc.tile_pool(name="psa", bufs=3, space="PSUM"))
            psb = p1.enter_context(tc.tile_pool(name="psb", bufs=4, space="PSUM"))
            for b in range(B):
                # one 4MB DMA per batch: [hp, (i, hb, w)]
                xt = xpool.tile([128, C, 2, 256], BF, tag="xt")
                nc.sync.dma_start(
                    xt[:],
                    xp[b].rearrange("i (hb hp) w -> hp i hb w", hb=2))
                for i2 in range(C // 2):
                    pa = psa.tile([128, 512], FP, tag="pa")
                    for j in range(2):
                        i = i2 * 2 + j
                        for ws in range(2):
                            for hb in range(2):
                                nc.tensor.matmul(
                                    pa[:, j * 256 + ws * 128:
                                       j * 256 + ws * 128 + 128],
                                    xt[:, i, hb, ws * 128: ws * 128 + 128],
                                    fh_sb[hb][:],
                                    start=(hb == 0), stop=(hb == 1))
                    zt = ztpool.tile([128, 512], BF, tag="zt")
                    if i2 % 2 == 0:
                        nc.vector.tensor_copy(zt[:], pa[:])
                    else:
                        nc.scalar.copy(zt[:], pa[:])
                    pb = psb.tile([64, 128], FP, tag="pb")
                    for j in range(2):
                        for ws in range(2):
                            z0 = j * 256 + ws * 128
                            nc.tensor.matmul(pb[:, j * 64:j * 64 + 64],
                                             zt[:, z0:z0 + 64],
                                             cwa_sb[ws][:],
                                             start=(ws == 0), stop=False)
                            nc.tensor.matmul(pb[:, j * 64:j * 64 + 64],
                                             zt[:, z0 + 64:z0 + 128],
                                             cwb_sb[ws][:],
                                             start=False, stop=(ws == 1))
                    # pb cols (j2, c2, ky32) -> Fbig (kyh, kpar, i=2*i2+j, c)
                    fb8 = fbig[:].rearrange(
                        "p (b kyh kpar i2 j c) -> p b i2 kyh kpar j c",
                        b=B, kyh=M2 // 2, kpar=2, i2=C // 2, j=2)
                    pb8 = pb[:].rearrange(
                        "p (j c kyh kpar) -> p kyh kpar j c",
                        j=2, c=2, kyh=M2 // 2)
                    if i2 % 2 == 1:
                        nc.vector.tensor_copy(fb8[:, b, i2], pb8)
                    else:
                        nc.scalar.copy(fb8[:, b, i2], pb8)

        # ================= Phase T1: F -> FT transpose =================
        # transpose chunks [64 kx, 128 (kpar, i, c)] -> [128, 64 kx]
        ft5 = ft[:].rearrange("p (kx kyh b) -> p kx kyh b",
                              kx=KX, kyh=M2 // 2, b=B)
        with ExitStack() as pt1:
            pst = pt1.enter_context(tc.tile_pool(name="pst", bufs=4, space="PSUM"))
            for b in range(B):
                for kyh in range(M2 // 2):
                    c0 = b * 2048 + kyh * 128
                    pt = pst.tile([128, 64], BF, tag="pt")
                    nc.tensor.transpose(pt[:], fbig[:, c0:c0 + 128], id64[:])
                    if kyh % 2 == 0:
                        nc.vector.tensor_copy(ft5[:, :, kyh, b], pt[:])
                    else:
                        nc.scalar.copy(ft5[:, :, kyh, b], pt[:])

        # ================= Phase MIX =================
        # per mode (kx, ky = 2*kyh + kpar): stationary embedding [64 (i,c),
        # 64 (rF,o)] at partition half kpar*64, rhs ft rows kpar*64.. ->
        # 2 modes run on disjoint PE quadrants concurrently
        p45 = p4[:].rearrange("p (kyh b kx) -> p kyh b kx",
                              kyh=M2 // 2, b=B, kx=KX)
        with ExitStack() as pm:
            wpool = pm.enter_context(tc.tile_pool(name="w", bufs=2))
            psm = pm.enter_context(tc.tile_pool(name="psm", bufs=4, space="PSUM"))
            for q in range(16):  # kx quad
                wt = wpool.tile([128, 4096], BF, tag="wt")
                nc.sync.dma_start(wt[:], wm[q])
                for kxs in range(4):
                    kx = q * 4 + kxs
                    pm_t = psm.tile([128, 64], FP, tag="pmix")
                    for kyh in range(M2 // 2):
                        for kpar in range(2):
                            nc.tensor.matmul(
                                pm_t[kpar * 64:kpar * 64 + 64,
                                     kyh * 4:kyh * 4 + 4],
                                wt[kpar * 64:kpar * 64 + 64,
                                   kxs * 1024 + kyh * 64:
                                   kxs * 1024 + kyh * 64 + 64],
                                ft[kpar * 64:kpar * 64 + 64,
                                   kx * 64 + kyh * 4:kx * 64 + kyh * 4 + 4],
                                start=True, stop=True)
                    # psum rows (kpar, rF, o), cols (kyh, b) -> P4 col
                    if kx % 2 == 0:
                        nc.vector.tensor_copy(p45[:, :, :, kx], pm_t[:])
                    else:
                        nc.scalar.copy(p45[:, :, :, kx], pm_t[:])

        # ================= Phase T2 + D =================
        # T2: transpose [128 (kpar,rF,o), 64 kx] chunks of p4 ->
        #   td rows kx, cols (kyh, kpar, rF, b, o)
        td4 = td[:].rearrange("p (kyh kpar rf b o) -> p kyh b kpar rf o",
                              kyh=M2 // 2, kpar=2, rf=2, b=B)
        with ExitStack() as pt2:
            pst2 = pt2.enter_context(tc.tile_pool(name="pst2", bufs=4, space="PSUM"))
            for kyh in range(M2 // 2):
                for b in range(B):
                    c0 = kyh * 256 + b * 64
                    pt = pst2.tile([64, 128], BF, tag="pt2")
                    nc.tensor.transpose(pt[:], p4[:, c0:c0 + 64], id128[:])
                    if b % 2 == 0:
                        nc.vector.tensor_copy(td4[:, kyh, b], pt[:])
                    else:
                        nc.scalar.copy(td4[:, kyh, b], pt[:])
        with ExitStack() as pd_s:
            psd = pd_s.enter_context(tc.tile_pool(name="psd", bufs=3, space="PSUM"))
            # U cols: hh*128 + hp*64 + ky*2 + rU ; psum cols (hh, hp, rU)
            u4 = u_sb[:].rearrange("p (hh hp ky ru) -> p hh hp ru ky",
                                   hh=128, hp=2, ky=M2)
            for ky in range(M2):
                kyh, kpar = ky // 2, ky % 2
                pd = psd.tile([128, 512], FP, tag="pd")
                g0 = kyh * 512 + kpar * 256
                nc.tensor.matmul(pd[:], td[:, g0:g0 + 128],
                                 ehre_sb[:], start=True, stop=False)
                nc.tensor.matmul(pd[:], td[:, g0 + 128:g0 + 256],
                                 ehim_sb[:], start=False, stop=True)
                if ky % 2 == 0:
                    nc.vector.tensor_copy(u4[:, :, :, :, ky], pd[:])
                else:
                    nc.scalar.copy(u4[:, :, :, :, ky], pd[:])

        # ================= Phase T3 + E + output =================
        yb = yp.rearrange("b o h w -> (b o) h w")
        with ExitStack() as pe_s:
            pst3 = pe_s.enter_context(tc.tile_pool(name="pst3", bufs=3, space="PSUM"))
            utp = pe_s.enter_context(tc.tile_pool(name="ut", bufs=3))
            pse = pe_s.enter_context(tc.tile_pool(name="pse", bufs=4, space="PSUM"))
            ostp = pe_s.enter_context(tc.tile_pool(name="ost", bufs=3))
            ost = None
            for hh in range(128):
                pt = pst3.tile([128, 128], BF, tag="pt3")
                nc.tensor.transpose(pt[:], u_sb[:, hh * 128:(hh + 1) * 128],
                                    id128[:])
                ut = utp.tile([128, 128], BF, tag="ut")
                if hh % 2 == 0:
                    nc.vector.tensor_copy(ut[:], pt[:])
                else:
                    nc.scalar.copy(ut[:], pt[:])
                if hh % 4 == 0:
                    ost = ostp.tile([128, 2048], F16, tag="ost")
                # ut rows (hp, ky, rU); EW3 block-diagonal in hp
                # out pe cols (hp, w) = rows h = 2*hh + hp
                pe = pse.tile([128, 512], FP, tag="pe")
                nc.tensor.matmul(pe[:], ut[:], ew_sb[:],
                                 start=True, stop=True)
                if hh % 2 == 0:
                    nc.scalar.copy(
                        ost[:, (hh % 4) * 512:(hh % 4) * 512 + 512], pe[:])
                else:
                    nc.vector.tensor_copy(
                        ost[:, (hh % 4) * 512:(hh % 4) * 512 + 512], pe[:])
                if hh % 4 == 3:
                    nc.sync.dma_start(
                        yb[:, (hh - 3) * 2:(hh + 1) * 2, :], ost[:])

    nc.compile()
    return nc


_NC = None


def kernel(x, w1_re, w1_im, w4_re, w4_im):
    global _NC
    if _NC is None:
        _NC = _build()
    consts = _constants()
    in_maps = []
    for p in range(P):
        # pack MIX embeddings:
        #   [q, kpar*64 + i*2 + c, kxs*1024 + kyh*64 + rf*32 + o]
        # rows c=0 (Fre): [Wre | Wim]; rows c=1 (Fim): [-Wim | Wre]
        wre = np.concatenate([w1_re[:, :, p], w4_re[:, :, p]], axis=2)
        wim = np.concatenate([w1_im[:, :, p], w4_im[:, :, p]], axis=2)
        emb = np.empty((2, 2, C, C, KX, M2), np.float32)  # [c, rf, i, o, ...]
        emb[0, 0] = wre
        emb[0, 1] = wim
        emb[1, 0] = -wim
        emb[1, 1] = wre
        emb = emb.reshape(2, 2, C, C, 16, 4, 16, 2)  # c rf i o q kxs kyh kpar
        emb = emb.transpose(4, 7, 2, 0, 5, 6, 1, 3)  # q kpar i c kxs kyh rf o
        wmp = np.ascontiguousarray(emb.reshape(16, 128, 4096)).astype(BFNP)
        m = {
            "xp": np.ascontiguousarray(x[p]).astype(BFNP),
            "wm": wmp,
        }
        m.update(consts)
        in_maps.append(m)
    res = run_bass_kernel_spmd(_NC, in_maps, core_ids=list(range(P)))
    return np.stack([res.results[p]["yp"] for p in range(P)],
                    axis=0).astype(np.float32)


if __name__ == "__main__":
    rng = np.random.default_rng(0)
    x = rng.standard_normal((P, B, C, H, W)).astype(np.float32)
    wshape = (C, C, P, M1, M2)
    ws = [(rng.random(wshape, np.float32) / (C * C)).astype(np.float32)
          for _ in range(4)]
    out = kernel(x, *ws)
    print("out", out.shape, out.dtype, float(np.abs(out).max()))



# revision 46
# speedup vs baseline: 1.7938x; 1.0409x over previous
"""Trainium2 Bass kernel for nn_GNO2d (spectral conv, method-25 branch).

Sharded over pipes P=8, one pipe per NeuronCore. Per pipe the computation is a
truncated 2-D rFFT -> per-mode complex channel mixing -> inverse rFFT,
implemented entirely as TensorEngine matmuls against small DFT constant
matrices (bf16 operands, fp32 PSUM accumulation):

  A: Z[kx,w]   = sum_h x[h,w] e^{-i th_kx h}          (64 retained kx rows)
  B: F[kx,ky]  = sum_w Z[kx,w] e^{-i ph_ky w}         (32 retained ky cols)
  T1: PE-transpose F from [kx,..] to [(rW,i),..] layout
  MIX: f[o,..] = sum_i W[i,o] F[i,..]  (complex, via K=(re/im,i)=64 matmuls)
  T2: PE-transpose f to [(kx,rF),..] layout
  D: U[.,h]    = sum_{kx,rF} f e^{+i th h}            (complex combine folded
  T3: PE-transpose U to [(hp,ky,rU),..] layout          into constant matrices)
  E: y[.,w]    = Re sum_{ky,rU} c_ky U e^{+i ph w} / (H W)
"""

import numpy as np
import ml_dtypes
from contextlib import ExitStack

import concourse.bass as bass
import concourse.tile as tile
import concourse.mybir as mybir
from concourse import bacc
from concourse.bass_utils import run_bass_kernel_spmd

P, B, C, H, W = 8, 4, 32, 256, 256
M1, M2 = 32, 32
KX = 2 * M1  # 64 retained kx rows
FP = mybir.dt.float32
BF = mybir.dt.bfloat16
BFNP = ml_dtypes.bfloat16


def _constants():
    """Host-side DFT constant matrices, bf16."""
    freqs = np.concatenate([np.arange(M1), np.arange(H - M1, H)])  # kx freqs
    th = 2 * np.pi * np.outer(np.arange(H), freqs) / H             # [H, KX]
    phi = 2 * np.pi * np.outer(np.arange(W), np.arange(M2)) / W    # [W, M2]

    fh = np.concatenate([np.cos(th), -np.sin(th)], axis=1)         # [256, 128]
    cwa = np.concatenate([np.cos(phi), -np.sin(phi)], axis=1)      # [256, 64]
    cwb = np.concatenate([np.sin(phi), np.cos(phi)], axis=1)       # [256, 64]

    # EHRE/EHIM rows kx, cols (hh*4 + hp*2 + rU)  [h = hh*2 + hp]
    # U_ru = sum_kx Gre*(ru? sin:cos) + Gim*(ru? cos:-sin)
    c, s = np.cos(th.T), np.sin(th.T)                              # [KX, H]
    ehre = np.stack([c, s], axis=2).reshape(64, 512)       # cols (h, rU)
    ehim = np.stack([-s, c], axis=2).reshape(64, 512)
    # (h, rU) with h=(hh,hp) -> (hh, hp, rU) is already the natural order
    # since h*2+ru = hh*4 + hp*2 + ru

    # EW3 rows (hp*64 + ky*2 + rU), cols (hp'*256 + w), block-diagonal in hp
    cky = np.where(np.arange(M2) == 0, 1.0, 2.0)[:, None]
    ewc = cky * np.cos(phi.T) / (H * W)                            # [M2, W]
    ews = cky * np.sin(phi.T) / (H * W)
    ew = np.zeros((128, 512), np.float32)
    for hp in range(2):
        ew[hp * 64:hp * 64 + 64:2, hp * 256:hp * 256 + 256] = ewc
        ew[hp * 64 + 1:hp * 64 + 64:2, hp * 256:hp * 256 + 256] = -ews

    consts = {
        "FH": fh, "CWA": cwa, "CWB": cwb,
        "EHRE": ehre, "EHIM": ehim, "EW3": ew,
        "ID64": np.eye(64, dtype=np.float32),
        "ID128": np.eye(128, dtype=np.float32),
    }
    return {k: np.ascontiguousarray(v.astype(BFNP)) for k, v in consts.items()}


F16 = mybir.dt.float16


def _build():
    nc = bacc.Bacc("TRN2", target_bir_lowering=False, debug=False, num_devices=P)
    xp = nc.dram_tensor("xp", [B, C, H, W], BF, kind="ExternalInput").ap()
    # host-packed MIX embeddings:
    #   [q, kpar*64 + i*2 + c, kxs*1024 + kyh*64 + rf*32 + o]
    wm = nc.dram_tensor("wm", [16, 128, 4096], BF, kind="ExternalInput").ap()
    cdr = {}
    for name, shape in [("FH", [256, 128]), ("CWA", [256, 64]), ("CWB", [256, 64]),
                        ("EHRE", [64, 512]), ("EHIM", [64, 512]),
                        ("EW3", [128, 512]),
                        ("ID64", [64, 64]), ("ID128", [128, 128])]:
        cdr[name] = nc.dram_tensor(name, shape, BF, kind="ExternalInput").ap()
    yp = nc.dram_tensor("yp", [B, C, H, W], F16, kind="ExternalOutput").ap()

    with tile.TileContext(nc) as tc, ExitStack() as ctx:
        # -------- constants to SBUF --------
        cp = ctx.enter_context(tc.tile_pool(name="consts", bufs=1))
        fh_sb, cwa_sb, cwb_sb = [], [], []
        for hb in range(2):
            t = cp.tile([128, 128], BF, tag=f"fh{hb}")
            nc.sync.dma_start(t[:], cdr["FH"][hb * 128:(hb + 1) * 128, :])
            fh_sb.append(t)
            ta = cp.tile([128, 64], BF, tag=f"cwa{hb}")
            nc.sync.dma_start(ta[:], cdr["CWA"][hb * 128:(hb + 1) * 128, :])
            cwa_sb.append(ta)
            tb = cp.tile([128, 64], BF, tag=f"cwb{hb}")
            nc.sync.dma_start(tb[:], cdr["CWB"][hb * 128:(hb + 1) * 128, :])
            cwb_sb.append(tb)
        ehre_sb = cp.tile([64, 512], BF, tag="ehre")
        nc.sync.dma_start(ehre_sb[:], cdr["EHRE"][:])
        ehim_sb = cp.tile([64, 512], BF, tag="ehim")
        nc.sync.dma_start(ehim_sb[:], cdr["EHIM"][:])
        ew_sb = cp.tile([128, 512], BF, tag="ew")
        nc.sync.dma_start(ew_sb[:], cdr["EW3"][:])
        id64 = cp.tile([64, 64], BF, tag="id64")
        nc.sync.dma_start(id64[:], cdr["ID64"][:])
        id128 = cp.tile([128, 128], BF, tag="id128")
        nc.sync.dma_start(id128[:], cdr["ID128"][:])

        # -------- persistent intermediate tensors --------
        big = ctx.enter_context(tc.tile_pool(name="big", bufs=1))
        # Fbig cols: b*2048 + kyh*128 + kpar*64 + i*2 + c ; rows kx64
        fbig = big.tile([64, 8192], BF, tag="fbig")
        # FT cols: kx*64 + kyh*4 + b ; rows (kpar2, i32, c2)
        ft = big.tile([128, 4096], BF, tag="ft")
        # P4 cols: kyh*256 + b*64 + kx ; rows (kpar2, rF2, o32)
        p4 = big.tile([128, 4096], BF, tag="p4")
        # TD cols: kyh*512 + kpar*256 + rF*128 + b*32 + o ; rows kx64
        td = big.tile([64, 8192], BF, tag="td")
        # U cols: hh*128 + hp*64 + ky*2 + rU ; rows (b4, o32)
        u_sb = big.tile([128, 16384], BF, tag="u")

        # ================= Phase 1: stages A + B =================
        with ExitStack() as p1:
            xpool = p1.enter_context(tc.tile_pool(name="x", bufs=6))
            ztpool = p1.enter_context(tc.tile_pool(name="zt", bufs=6))
            psa = p1.enter_context(tc.tile_pool(name="psa", bufs=4, space="PSUM"))
            psb = p1.enter_context(tc.tile_pool(name="psb", bufs=4, space="PSUM"))
            for b in range(B):
                # four 1MB DMAs per batch (i-octs): [hp, (i8, hb, w)]
                xts = []
                for io in range(4):
                    xt = xpool.tile([128, 8, 2, 256], BF, tag="xt")
                    nc.sync.dma_start(
                        xt[:],
                        xp[b, io * 8:(io + 1) * 8].rearrange(
                            "i (hb hp) w -> hp i hb w", hb=2))
                    xts.append(xt)
                for i2 in range(C // 2):
                    pa = psa.tile([128, 512], FP, tag="pa")
                    for j in range(2):
                        i = i2 * 2 + j
                        xt = xts[i // 8]
                        for ws in range(2):
                            for hb in range(2):
                                nc.tensor.matmul(
                                    pa[:, j * 256 + ws * 128:
                                       j * 256 + ws * 128 + 128],
                                    xt[:, i % 8, hb, ws * 128: ws * 128 + 128],
                                    fh_sb[hb][:],
                                    start=(hb == 0), stop=(hb == 1))
                    zt = ztpool.tile([128, 512], BF, tag="zt")
                    if i2 % 2 == 0:
                        nc.vector.tensor_copy(zt[:], pa[:])
                    else:
                        nc.scalar.copy(zt[:], pa[:])
                    pb = psb.tile([64, 128], FP, tag="pb")
                    for j in range(2):
                        for ws in range(2):
                            z0 = j * 256 + ws * 128
                            nc.tensor.matmul(pb[:, j * 64:j * 64 + 64],
                                             zt[:, z0:z0 + 64],
                                             cwa_sb[ws][:],
                                             start=(ws == 0), stop=False)
                            nc.tensor.matmul(pb[:, j * 64:j * 64 + 64],
                                             zt[:, z0 + 64:z0 + 128],
                                             cwb_sb[ws][:],
                                             start=False, stop=(ws == 1))
                    # pb cols (j2, c2, ky32) -> Fbig (kyh, kpar, i=2*i2+j, c)
                    fb8 = fbig[:].rearrange(
                        "p (b kyh kpar i2 j c) -> p b i2 kyh kpar j c",
                        b=B, kyh=M2 // 2, kpar=2, i2=C // 2, j=2)
                    pb8 = pb[:].rearrange(
                        "p (j c kyh kpar) -> p kyh kpar j c",
                        j=2, c=2, kyh=M2 // 2)
                    if i2 % 2 == 1:
                        nc.vector.tensor_copy(fb8[:, b, i2], pb8)
                    else:
                        nc.scalar.copy(fb8[:, b, i2], pb8)

        # ================= Phase T1: F -> FT transpose =================
        # transpose chunks [64 kx, 128 (kpar, i, c)] -> [128, 64 kx]
        ft5 = ft[:].rearrange("p (kx kyh b) -> p kx kyh b",
                              kx=KX, kyh=M2 // 2, b=B)
        with ExitStack() as pt1:
            pst = pt1.enter_context(tc.tile_pool(name="pst", bufs=4, space="PSUM"))
            for b in range(B):
                for kyh in range(M2 // 2):
                    c0 = b * 2048 + kyh * 128
                    pt = pst.tile([128, 64], BF, tag="pt")
                    nc.tensor.transpose(pt[:], fbig[:, c0:c0 + 128], id64[:])
                    if kyh % 2 == 0:
                        nc.vector.tensor_copy(ft5[:, :, kyh, b], pt[:])
                    else:
                        nc.scalar.copy(ft5[:, :, kyh, b], pt[:])

        # ================= Phase MIX =================
        # per mode (kx, ky = 2*kyh + kpar): stationary embedding [64 (i,c),
        # 64 (rF,o)] at partition half kpar*64, rhs ft rows kpar*64.. ->
        # 2 modes run on disjoint PE quadrants concurrently
        p45 = p4[:].rearrange("p (kyh b kx) -> p kyh b kx",
                              kyh=M2 // 2, b=B, kx=KX)
        with ExitStack() as pm:
            wpool = pm.enter_context(tc.tile_pool(name="w", bufs=3))
            psm = pm.enter_context(tc.tile_pool(name="psm", bufs=4, space="PSUM"))
            for q in range(16):  # kx quad
                wt = wpool.tile([128, 4096], BF, tag="wt")
                nc.sync.dma_start(wt[:], wm[q])
                for kxs in range(4):
                    kx = q * 4 + kxs
                    pm_t = psm.tile([128, 64], FP, tag="pmix")
                    for kyh in range(M2 // 2):
                        for kpar in range(2):
                            nc.tensor.matmul(
                                pm_t[kpar * 64:kpar * 64 + 64,
                                     kyh * 4:kyh * 4 + 4],
                                wt[kpar * 64:kpar * 64 + 64,
                                   kxs * 1024 + kyh * 64:
                                   kxs * 1024 + kyh * 64 + 64],
                                ft[kpar * 64:kpar * 64 + 64,
                                   kx * 64 + kyh * 4:kx * 64 + kyh * 4 + 4],
                                start=True, stop=True)
                    # psum rows (kpar, rF, o), cols (kyh, b) -> P4 col
                    if kx % 2 == 0:
                        nc.vector.tensor_copy(p45[:, :, :, kx], pm_t[:])
                    else:
                        nc.scalar.copy(p45[:, :, :, kx], pm_t[:])

        # ================= Phase T2 + D =================
        # T2: transpose [128 (kpar,rF,o), 64 kx] chunks of p4 ->
        #   td rows kx, cols (kyh, kpar, rF, b, o)
        td4 = td[:].rearrange("p (kyh kpar rf b o) -> p kyh b kpar rf o",
                              kyh=M2 // 2, kpar=2, rf=2, b=B)
        with ExitStack() as pt2:
            pst2 = pt2.enter_context(tc.tile_pool(name="pst2", bufs=4, space="PSUM"))
            for kyh in range(M2 // 2):
                for b in range(B):
                    c0 = kyh * 256 + b * 64
                    pt = pst2.tile([64, 128], BF, tag="pt2")
                    nc.tensor.transpose(pt[:], p4[:, c0:c0 + 64], id128[:])
                    if b % 2 == 0:
                        nc.vector.tensor_copy(td4[:, kyh, b], pt[:])
                    else:
                        nc.scalar.copy(td4[:, kyh, b], pt[:])
        with ExitStack() as pd_s:
            psd = pd_s.enter_context(tc.tile_pool(name="psd", bufs=3, space="PSUM"))
            # U cols: hh*128 + hp*64 + ky*2 + rU ; psum cols (hh, hp, rU)
            u4 = u_sb[:].rearrange("p (hh hp ky ru) -> p hh hp ru ky",
                                   hh=128, hp=2, ky=M2)
            for ky in range(M2):
                kyh, kpar = ky // 2, ky % 2
                pd = psd.tile([128, 512], FP, tag="pd")
                g0 = kyh * 512 + kpar * 256
                nc.tensor.matmul(pd[:], td[:, g0:g0 + 128],
                                 ehre_sb[:], start=True, stop=False)
                nc.tensor.matmul(pd[:], td[:, g0 + 128:g0 + 256],
                                 ehim_sb[:], start=False, stop=True)
                if ky % 2 == 0:
                    nc.vector.tensor_copy(u4[:, :, :, :, ky], pd[:])
                else:
                    nc.scalar.copy(u4[:, :, :, :, ky], pd[:])

        # ================= Phase T3 + E + output =================
        yb = yp.rearrange("b o h w -> (b o) h w")
        with ExitStack() as pe_s:
            pst3 = pe_s.enter_context(tc.tile_pool(name="pst3", bufs=3, space="PSUM"))
            utp = pe_s.enter_context(tc.tile_pool(name="ut", bufs=3))
            pse = pe_s.enter_context(tc.tile_pool(name="pse", bufs=4, space="PSUM"))
            ostp = pe_s.enter_context(tc.tile_pool(name="ost", bufs=3))
            ost = None
            for hh in range(128):
                pt = pst3.tile([128, 128], BF, tag="pt3")
                nc.tensor.transpose(pt[:], u_sb[:, hh * 128:(hh + 1) * 128],
                                    id128[:])
                ut = utp.tile([128, 128], BF, tag="ut")
                if hh % 2 == 0:
                    nc.vector.tensor_copy(ut[:], pt[:])
                else:
                    nc.scalar.copy(ut[:], pt[:])
                if hh % 4 == 0:
                    ost = ostp.tile([128, 2048], F16, tag="ost")
                # ut rows (hp, ky, rU); EW3 block-diagonal in hp
                # out pe cols (hp, w) = rows h = 2*hh + hp
                pe = pse.tile([128, 512], FP, tag="pe")
                nc.tensor.matmul(pe[:], ut[:], ew_sb[:],
                                 start=True, stop=True)
                if hh % 2 == 0:
                    nc.scalar.copy(
                        ost[:, (hh % 4) * 512:(hh % 4) * 512 + 512], pe[:])
                else:
                    nc.vector.tensor_copy(
                        ost[:, (hh % 4) * 512:(hh % 4) * 512 + 512], pe[:])
                if hh % 4 == 3:
                    nc.sync.dma_start(
                        yb[:, (hh - 3) * 2:(hh + 1) * 2, :], ost[:])

    nc.compile()
    return nc


_NC = None


def kernel(x, w1_re, w1_im, w4_re, w4_im):
    global _NC
    if _NC is None:
        _NC = _build()
    consts = _constants()
    in_maps = []
    for p in range(P):
        # pack MIX embeddings:
        #   [q, kpar*64 + i*2 + c, kxs*1024 + kyh*64 + rf*32 + o]
        # rows c=0 (Fre): [Wre | Wim]; rows c=1 (Fim): [-Wim | Wre]
        wre = np.concatenate([w1_re[:, :, p], w4_re[:, :, p]], axis=2)
        wim = np.concatenate([w1_im[:, :, p], w4_im[:, :, p]], axis=2)
        emb = np.empty((2, 2, C, C, KX, M2), np.float32)  # [c, rf, i, o, ...]
        emb[0, 0] = wre
        emb[0, 1] = wim
        emb[1, 0] = -wim
        emb[1, 1] = wre
        emb = emb.reshape(2, 2, C, C, 16, 4, 16, 2)  # c rf i o q kxs kyh kpar
        emb = emb.transpose(4, 7, 2, 0, 5, 6, 1, 3)  # q kpar i c kxs kyh rf o
        wmp = np.ascontiguousarray(emb.reshape(16, 128, 4096)).astype(BFNP)
        m = {
            "xp": np.ascontiguousarray(x[p]).astype(BFNP),
            "wm": wmp,
        }
        m.update(consts)
        in_maps.append(m)
    res = run_bass_kernel_spmd(_NC, in_maps, core_ids=list(range(P)))
    return np.stack([res.results[p]["yp"] for p in range(P)],
                    axis=0).astype(np.float32)


if __name__ == "__main__":
    rng = np.random.default_rng(0)
    x = rng.standard_normal((P, B, C, H, W)).astype(np.float32)
    wshape = (C, C, P, M1, M2)
    ws = [(rng.random(wshape, np.float32) / (C * C)).astype(np.float32)
          for _ in range(4)]
    out = kernel(x, *ws)
    print("out", out.shape, out.dtype, float(np.abs(out).max()))

